# revision 18
# baseline (speedup 1.0000x reference)
"""Distributed causal multi-head attention for TRN2 (8 NeuronCores).

Sharding: tensor-parallel over heads — core c owns heads {2c, 2c+1} for both
batches. QKV projections computed in transposed layout (feature on partitions,
tokens on free axis), attention computed as S.T = K @ Q.T per 128-key block
with softmax denominators obtained by augmenting V with a ones column. Five
segment-split AllToAlls re-shard from head-parallel to token-parallel as
attention progresses; each core then applies the output projection for its
4 x 128 tokens.

Differences from the v1 schedule (hardware-measured rationale):
- y is normalized on the SEND side: the eviction computes 1/den locally,
  broadcasts it across the 64 v-dim partitions (gpsimd ucode op), and the
  eviction copy becomes a multiply. The A2A ships normalized y (128 rows per
  chunk, no denominator row), and the post-collective path is load -> output
  projection only — no DVE op anywhere downstream of a collective.
- Head: the first-needed weight halves and x chunks fan out over 4 queues
  (sync/gpsimd/scalar/vector) so the first qkv matmul starts ~6us, not ~15.
- QKV for tokens 1024+ runs in 1024-token units (8 matmuls of free=1024) and
  the output projection in 1024-free units: fewer instructions, same PSUM.
- Output rows for the two 64-token segments (b1 qc0 / b1 qc3) are adjacent
  (384..448, 448..512); their post work is merged into one 128-token group
  so the tail after the last A2A is 8 matmuls + 1 copy + 1 store.
- Attention pairs are interleaved within each q-chunk and evictions drain
  only their own pending PV jobs, so the PE keeps a PV backlog across
  segment transitions (HAM re-throttles after ~3.4us-thin windows).
"""

import sys

sys.path.insert(0, "/opt/trn_rl_repo")

import numpy as np
import ml_dtypes

import concourse.bacc as bacc
import concourse.bass as bass
import concourse.mybir as mybir
import concourse.tile as tile
from concourse.bass_utils import run_bass_kernel_spmd

BF16 = mybir.dt.bfloat16
F32 = mybir.dt.float32
NPBF16 = ml_dtypes.bfloat16

B, T, C, H, D = 2, 2048, 1024, 16, 64
NCORES = 8
HPC = H // NCORES          # heads per core = 2
CP = HPC * D               # feature columns per core = 128
TF = B * T                 # flat tokens = 4096
TS = TF // NCORES          # output tokens per core = 512
# segments: (batch, qcs, tokens-per-core); b1's qc0 gets its own small A2A
# that triggers ~20us before attention ends, so the final barrier carries
# only qc3 and its post work is merged with qc0's (adjacent output rows)
SEGS = [
    (0, (0, 1), 128),
    (0, (2, 3), 128),
    (1, (1, 2), 128),
    (1, (0,), 64),
    (1, (3,), 64),
]
NSEG = len(SEGS)
OUTOFF = [0, 128, 256, 384, 448]
SEG_OF = {}
for _g, (_b, _qcs, _tps) in enumerate(SEGS):
    for _i, _qc in enumerate(_qcs):
        SEG_OF[(_b, _qc)] = (_g, _i, _tps)
NCB = C // 128             # feature blocks = 8
NQC = T // 512             # q-chunks per batch = 4
NKB = T // 128             # key blocks per batch = 16
SCALE = float(D) ** -0.5
MASKVAL = -30000.0
CH = 128                   # a2a chunk rows: 64 per head half, normalized y


def build_nc():
    nc = bacc.Bacc("TRN2", target_bir_lowering=False, num_devices=NCORES)

    xT = nc.dram_tensor("xT", [C, TF], BF16, kind="ExternalInput")
    # weights pre-packed on host to the on-chip layout [128, NCB, blockcols]
    wqT = nc.dram_tensor("wqT", [128, NCB * CP], BF16, kind="ExternalInput")
    wkT = nc.dram_tensor("wkT", [128, NCB * CP], BF16, kind="ExternalInput")
    wvT = nc.dram_tensor("wvT", [128, NCB * CP], BF16, kind="ExternalInput")
    woT = nc.dram_tensor("woT", [128, NCB * C], BF16, kind="ExternalInput")
    mtri = nc.dram_tensor("mtri", [128, 128], BF16, kind="ExternalInput")
    ident = nc.dram_tensor("ident", [128, 64], BF16, kind="ExternalInput")
    out = nc.dram_tensor("out", [TS, C], BF16, kind="ExternalOutput")

    with tile.TileContext(nc) as tc:
        with (
            tc.tile_pool(name="consts", bufs=1) as consts,
            tc.tile_pool(name="xp", bufs=1) as xp,
            tc.tile_pool(name="qkv", bufs=1) as qkv,
            tc.tile_pool(name="work", bufs=1) as work,
            tc.tile_pool(name="ps", bufs=1, space="PSUM") as psp,
            tc.tile_pool(name="dram", bufs=1, space="DRAM") as dram,
        ):
            # ---- weights & constants ----
            wq_sb = consts.tile([128, NCB, CP], BF16)
            wk_sb = consts.tile([128, NCB, CP], BF16)
            wv_sb = consts.tile([128, NCB, CP], BF16)
            wo_sb = consts.tile([128, NCB, C], BF16)
            mtri_sb = consts.tile([128, 128], BF16)
            ident_sb = consts.tile([128, 64], BF16)
            x_sb = [xp.tile([128, TF], BF16, name=f"x_sb{cb}") for cb in range(NCB)]
            hw_ = NCB // 2

            def wdma(eng, w_t, w_d, lo, hi):
                eng.dma_start(w_t[:, lo:hi, :], w_d[:, lo * CP : hi * CP])

            def xdma(eng, cb, t0, t1):
                eng.dma_start(x_sb[cb][:, t0:t1], xT[128 * cb : 128 * cb + 128, t0:t1])

            # Only sync/scalar (HWDGE) and gpsimd issue DMAs. Per-ring
            # transfers serialize at ~50GB/s, so chunks are placed by
            # need-time: wave0 (tokens 0-511) + wq/wk feed the prologue,
            # wave1 (512-1023) the q1 units (~15us), wave2 (1024-2047) the
            # q2 units (~28us), wave3a/b (b1 halves) the q4/q6 units
            # (~75/110us), wo the tail (~190us). The sync ring carries only
            # ~0.9MB so eviction DMAs from ~30us are never queued behind bulk.
            _q3 = (nc.sync, nc.gpsimd, nc.scalar)
            nc.scalar.dma_start(mtri_sb[:], mtri[:])
            nc.gpsimd.dma_start(ident_sb[:], ident[:])
            wdma(nc.sync, wq_sb, wqT, 0, hw_)
            wdma(nc.gpsimd, wq_sb, wqT, hw_, NCB)
            wdma(nc.scalar, wk_sb, wkT, 0, hw_)
            for cb in range(NCB):
                xdma(_q3[cb % 3], cb, 0, 512)
            wdma(nc.scalar, wk_sb, wkT, hw_, NCB)
            # Wave 1: wv + x tokens 512-1023
            wdma(nc.sync, wv_sb, wvT, 0, hw_)
            wdma(nc.gpsimd, wv_sb, wvT, hw_, NCB)
            for cb in range(NCB):
                xdma(_q3[cb % 3], cb, 512, 1024)
            # Wave 2: b0 second half in 512-token chunks, 3-way round-robin
            # (needed ~28us — a 2-ring split lands too late)
            for tcn in (2, 3):
                for cb in range(NCB):
                    xdma(_q3[cb % 3], cb, 512 * tcn, 512 * tcn + 512)
            # Wave 3: b1 in two 512-token halves per block so the q4 units
            # gate only on the first half; 3-way (eviction DMAs are small
            # enough that one 256KB chunk ahead of them costs ~1.5us)
            for cb in range(NCB):
                xdma(_q3[cb % 3], cb, 2048, 3072)
            for cb in range(NCB):
                xdma(_q3[(cb + 1) % 3], cb, 3072, 4096)
            # wo rides the scalar ring (its issues all precede the first exp,
            # and its transfers contend with nothing the PE waits on) so the
            # gpsimd ring is clear for the post y loads by ~85us
            nc.scalar.dma_start(wo_sb[:], woT[:])

            qT_sb = qkv.tile([128, TF], BF16)
            kT_sb = qkv.tile([128, TF], BF16)
            vT_sb = qkv.tile([128, TF], BF16)
            projs = ((wq_sb, qT_sb), (wk_sb, kT_sb), (wv_sb, vT_sb))

            v_sb = [work.tile([128, NKB, 65], BF16, name=f"v_sb{p}") for p in range(4)]

            # A2A groups: segs 2+3 share one collective (their evictions
            # complete back-to-back, and a separate seg3 A2A would serialize
            # behind seg2's on the CC stream, adding a full ~15us flight)
            GRP_W = [128, 128, 192, 64]        # chunk cols per group
            SEG_GRP = {0: (0, 0), 1: (1, 0), 2: (2, 0), 3: (2, 128), 4: (3, 0)}
            a2a_in = [
                dram.tile([NCORES * CH, GRP_W[g]], BF16, name=f"a2a_in{g}")
                for g in range(4)
            ]
            a2a_out = [
                dram.tile([NCORES * CH, GRP_W[g]], BF16, name=f"a2a_out{g}")
                for g in range(4)
            ]

            # ---------- emission units ----------
            def qkv_unit(tcn, pi):
                # 512-token units: each gates on exactly one x tranche DMA
                w_sb, oT = projs[pi]
                t0 = 512 * tcn
                ps = psp.tile(
                    [128, 512], F32, tag="st", bufs=3,
                    padded_shape=[128, 1024], name="ps_proj",
                )
                for cb in range(NCB):
                    nc.tensor.matmul(
                        ps[:],
                        lhsT=w_sb[:, cb, :],
                        rhs=x_sb[cb][:, t0 : t0 + 512],
                        start=(cb == 0),
                        stop=(cb == NCB - 1),
                    )
                if pi == 1:
                    nc.scalar.copy(oT[:, t0 : t0 + 512], ps[:])
                else:
                    nc.vector.tensor_copy(oT[:, t0 : t0 + 512], ps[:])

            def vt_unit(pair, kb):
                hh, b = pair % 2, pair // 2
                if kb == 0:
                    nc.vector.memset(v_sb[pair][:, :, 64:65], 1.0)
                t0 = 2048 * b + 128 * kb
                vt_ps = psp.tile([128, 64], BF16, tag="ot", bufs=2, name="vt_ps")
                nc.tensor.transpose(
                    vt_ps[:],
                    vT_sb[64 * hh : 64 * hh + 64, t0 : t0 + 128],
                    ident_sb[64 * hh : 64 * hh + 64, :],
                )
                nc.vector.tensor_copy(v_sb[pair][:, kb, 0:64], vt_ps[:])

            # attention state per (pair, qc), lives across kbp units
            attn_ot = {}
            pending_pv = []

            def emit_pv(job):
                pair, qc, kbp, pT, offs = job
                ot = attn_ot[(pair, qc)]
                n_kb = 4 * qc + 4
                for h2 in range(2):
                    kb = 2 * kbp + h2
                    off = offs[h2]
                    nc.tensor.matmul(
                        ot[:, off:512],
                        lhsT=v_sb[pair][:, kb, :],
                        rhs=pT[:, 512 * h2 + off : 512 * h2 + 512],
                        start=(kb == 0),
                        stop=(kb == n_kb - 1),
                    )

            def drain_pending(pair=None, qc=None):
                rest = []
                for job in pending_pv:
                    if pair is None or (job[0] == pair and job[1] == qc):
                        emit_pv(job)
                    else:
                        rest.append(job)
                pending_pv[:] = rest

            def attn_unit(pair, qc, kbp):
                hh, b = pair % 2, pair // 2
                hs = slice(64 * hh, 64 * hh + 64)
                tb0 = 2048 * b
                q0 = tb0 + 512 * qc
                if kbp == 0:
                    attn_ot[(pair, qc)] = psp.tile(
                        [65, 512], F32, tag="ot", bufs=2, name="ot_ps"
                    )
                st = psp.tile([128, 1024], F32, tag="st", bufs=3, name="st_ps")
                offs = []
                for h2 in range(2):
                    kb = 2 * kbp + h2
                    off = max(0, 128 * kb - 512 * qc)
                    offs.append(off)
                    nc.tensor.matmul(
                        st[:, 512 * h2 + off : 512 * h2 + 512],
                        lhsT=kT_sb[hs, tb0 + 128 * kb : tb0 + 128 * kb + 128],
                        rhs=qT_sb[hs, q0 + off : q0 + 512],
                        start=True,
                        stop=True,
                    )
                for h2 in range(2):
                    kb = 2 * kbp + h2
                    if 128 * kb >= 512 * qc:  # diagonal block: triangular mask
                        off = offs[h2]
                        dd = slice(512 * h2 + off, 512 * h2 + off + 128)
                        nc.vector.tensor_add(st[:, dd], st[:, dd], mtri_sb[:])
                pT = work.tile([128, 1024], BF16, tag="pT", bufs=8, name="pT")
                o0 = offs[0]
                nc.scalar.activation(
                    pT[:, o0:1024],
                    st[:, o0:1024],
                    mybir.ActivationFunctionType.Exp,
                    scale=SCALE,
                )
                pending_pv.append((pair, qc, kbp, pT, offs))
                if len(pending_pv) > 3:
                    emit_pv(pending_pv.pop(0))

            def evict_unit(pair, qc):
                drain_pending(pair, qc)
                hh, b = pair % 2, pair // 2
                g, qi, tps = SEG_OF[(b, qc)]
                nch = 512 // tps          # chunks this eviction covers
                s0 = nch * qi
                ot = attn_ot.pop((pair, qc))
                # send-side normalization: reciprocal of the denominator row,
                # broadcast across the 64 v-dims, folded into the eviction
                # copy. All inputs are local (PSUM), so nothing anywhere is
                # gated on a collective, and the post-A2A path needs no DVE.
                den32 = work.tile([1, 512], F32, tag="den32", bufs=3, name="den32")
                rec32 = work.tile([1, 512], F32, tag="rec32", bufs=3, name="rec32")
                recbc = work.tile([64, 512], F32, tag="recbc", bufs=2, name="recbc")
                y_sb = work.tile([64, 512], BF16, tag="y", bufs=12, name="y_sb")
                # stage through SBUF: the approx's bit-level seed needs raw
                # IEEE fp32, so don't feed it PSUM directly
                nc.vector.tensor_copy(den32[:], ot[64:65, :])
                nc.vector.reciprocal_approx_fast(rec32[:], den32[:])
                nc.gpsimd.partition_broadcast(recbc[:], rec32[:], channels=64)
                nc.vector.tensor_mul(y_sb[:], ot[0:64, :], recbc[:])
                grp, coff = SEG_GRP[g]
                w = GRP_W[grp]
                ydst = bass.AP(
                    a2a_in[grp].tensor,
                    (s0 * CH + 64 * hh) * w + coff,
                    [[w, 64], [CH * w, nch], [1, tps]],
                )
                nc.sync.dma_start(ydst, y_sb[:, :])

            def coll_unit(grp):
                nc.gpsimd.collective_compute(
                    "AllToAll",
                    mybir.AluOpType.bypass,
                    replica_groups=[list(range(NCORES))],
                    ins=[a2a_in[grp][:].opt()],
                    outs=[a2a_out[grp][:].opt()],
                )

            # post y tiles: segs 0-2 individually; segs 3+4 share one tile
            # (adjacent 64-token column halves -> one 128-token proj group)
            y_locs = {}

            def post_dma(g):
                # gpsimd queue: these loads gate on the seg's A2A completing
                tps = SEGS[g][2]
                grp, coff = SEG_GRP[g]
                w = GRP_W[grp]
                if g < 3:
                    y_loc = work.tile(
                        [128, NCB, tps], BF16, tag="yloc", bufs=3, name="y_loc"
                    )
                    dst0 = 0
                    y_locs[g] = y_loc
                else:
                    if 3 not in y_locs:
                        y_locs[3] = work.tile(
                            [128, NCB, 128], BF16, tag="yloc34", bufs=1, name="y_loc34"
                        )
                    y_loc = y_locs[3]
                    dst0 = 64 * (g - 3)
                for hh in range(2):  # partition rows 64*hh..64*hh+64
                    ysrc = bass.AP(
                        a2a_out[grp].tensor,
                        64 * hh * w + coff,
                        [[w, 64], [CH * w, NCB], [1, tps]],
                    )
                    nc.gpsimd.dma_start(
                        y_loc[64 * hh : 64 * hh + 64, :, dst0 : dst0 + tps], ysrc
                    )

            def proj_group(g):
                # g in {0,1,2}: 128 tokens at OUTOFF[g]; g==3: merged segs 3+4
                y_loc = y_locs[g]
                r0 = OUTOFF[g] if g < 3 else OUTOFF[3]
                ps = psp.tile(
                    [128, C], F32, tag="st", bufs=3,
                    padded_shape=[128, 1024], name="ps_op",
                )
                for mh in range(2):  # matmul out is one PSUM bank
                    for cb in range(NCB):
                        nc.tensor.matmul(
                            ps[:, 512 * mh : 512 * mh + 512],
                            lhsT=y_loc[:, cb, :],
                            rhs=wo_sb[:, cb, 512 * mh : 512 * mh + 512],
                            start=(cb == 0),
                            stop=(cb == NCB - 1),
                        )
                o_sb = work.tile([128, C], BF16, tag="osb", bufs=2, name="o_sb")
                nc.vector.tensor_copy(o_sb[:], ps[:])
                nc.sync.dma_start(out[r0 : r0 + 128, :], o_sb[:])

            def attn_units_for_seg(g):
                b, qcs, _ = SEGS[g]
                units = []
                for qc in qcs:
                    for hh in range(2):
                        pair = 2 * b + hh
                        for kbp in range(2 * qc + 2):
                            units.append(("a", pair, qc, kbp))
                        units.append(("e", pair, qc))
                return units

            def run_unit(u):
                if u[0] == "a":
                    attn_unit(u[1], u[2], u[3])
                elif u[0] == "e":
                    evict_unit(u[1], u[2])
                elif u[0] == "q":
                    qkv_unit(u[1], u[2])
                elif u[0] == "v":
                    vt_unit(u[1], u[2])

            feed = []
            fi = 0

            def run_chunk(main):
                nonlocal fi
                for u in main:
                    run_unit(u)
                    if fi < len(feed):
                        run_unit(feed[fi])
                        fi += 1

            def run_feed_one():
                nonlocal fi
                if fi < len(feed):
                    run_unit(feed[fi])
                    fi += 1

            def drain_feed():
                nonlocal fi
                while fi < len(feed):
                    run_unit(feed[fi])
                    fi += 1

            # ---------- schedule ----------
            # minimal prologue: only what attention (b0, qc0) needs up-front
            for pi in range(3):
                qkv_unit(0, pi)
            for hh in range(2):
                for kb in range(4):
                    vt_unit(hh, kb)

            feed += [("q", 1, pi) for pi in range(3)]
            feed += [("v", hh, kb) for hh in range(2) for kb in range(4, 8)]
            feed += [("q", 2, pi) for pi in range(3)]
            feed += [("q", 3, pi) for pi in range(3)]
            feed += [("v", hh, kb) for hh in range(2) for kb in range(8, NKB)]
            feed += [("q", 4, pi) for pi in range(3)]
            feed += [("q", 5, pi) for pi in range(3)]
            feed += [("v", 2 + hh, kb) for hh in range(2) for kb in range(8)]
            feed += [("q", 6, pi) for pi in range(3)]
            feed += [("q", 7, pi) for pi in range(3)]
            feed += [("v", 2 + hh, kb) for hh in range(2) for kb in range(8, 12)]
            # b1 v transposes for kb 12-15 are only needed by qc3 — they are
            # held back as PE filler for the thin b1-qc0 stretch below
            late_vt = [("v", 2 + hh, kb) for hh in range(2) for kb in range(12, NKB)]

            run_chunk(attn_units_for_seg(0))              # b0 qc0-1
            coll_unit(0)
            run_chunk(attn_units_for_seg(1))              # b0 qc2-3
            coll_unit(1)
            drain_feed()  # all b1 qkv/vt must be emitted before b1 attention
            main2 = attn_units_for_seg(2)                 # b1 qc1-2
            for i, u in enumerate(main2):
                if i == 20:
                    post_dma(0)
                run_unit(u)
            main3 = attn_units_for_seg(3)                 # b1 qc0 (6 units)
            vt_i = 0
            for i, u in enumerate(main3):
                if i == 2:
                    post_dma(1)
                run_unit(u)
                for _ in range(2):
                    if vt_i < len(late_vt):
                        run_unit(late_vt[vt_i])
                        vt_i += 1
                if i == 3:
                    # seg0's projection (inputs landed ~60us ago, nothing
                    # collective-gated) fills this thin stretch so HAM does
                    # not re-throttle right before qc3's attention
                    proj_group(0)
            coll_unit(2)  # merged segs 2+3 exchange
            main4 = attn_units_for_seg(4)                 # b1 qc3 (18 units)
            for i, u in enumerate(main4):
                if i == 2:
                    post_dma(2)
                if i == 4:
                    post_dma(3)
                run_unit(u)
            coll_unit(3)
            post_dma(4)
            # tail: held-back projections fill the final collective's flight
            # time; the seg3 half of the merged 3+4 group runs as soon as the
            # merged A2A lands, leaving only the seg4 half + copy + store
            # gated on the final collective
            proj_group(1)
            proj_group(2)
            y34 = y_locs[3]
            ps34 = psp.tile(
                [128, C], F32, tag="st", bufs=3,
                padded_shape=[128, 1024], name="ps34",
            )
            for half in range(2):  # 0: seg3 (gated merged A2A), 1: seg4
                for mh in range(2):
                    for cb in range(NCB):
                        nc.tensor.matmul(
                            ps34[64 * half : 64 * half + 64,
                                 512 * mh : 512 * mh + 512],
                            lhsT=y34[:, cb, 64 * half : 64 * half + 64],
                            rhs=wo_sb[:, cb, 512 * mh : 512 * mh + 512],
                            start=(cb == 0),
                            stop=(cb == NCB - 1),
                        )
            o34 = work.tile([128, C], BF16, tag="osb", bufs=2, name="o34")
            nc.vector.tensor_copy(o34[:], ps34[:])
            nc.sync.dma_start(out[OUTOFF[3] : OUTOFF[3] + 128, :], o34[:])

    nc.compile()
    return nc


_NC = None


def _get_nc():
    global _NC
    if _NC is None:
        _NC = build_nc()
    return _NC


def _host_consts():
    idx = np.arange(128)
    mtri = np.where(idx[None, :] >= idx[:, None], 0.0, MASKVAL).astype(np.float32)
    ident = np.concatenate([np.eye(64, dtype=np.float32)] * 2, axis=0)
    return mtri.astype(NPBF16), ident.astype(NPBF16)


def _pack_w(wT):
    # [C, cols] -> [128, NCB*cols]: w_sb[p, cb*cols+j] = wT[128*cb+p, j]
    cols = wT.shape[1]
    return np.ascontiguousarray(
        wT.reshape(NCB, 128, cols).transpose(1, 0, 2).reshape(128, NCB * cols)
    )


def _make_in_maps(x, Wq, Wk, Wv, Wo):
    xT = np.ascontiguousarray(x.reshape(TF, C).T).astype(NPBF16)
    woT = _pack_w(Wo.T).astype(NPBF16)
    mtri, ident = _host_consts()
    in_maps = []
    for c in range(NCORES):
        rows = slice(CP * c, CP * c + CP)
        in_maps.append(
            {
                "xT": xT,
                "wqT": _pack_w(Wq[rows].T).astype(NPBF16),
                "wkT": _pack_w(Wk[rows].T).astype(NPBF16),
                "wvT": _pack_w(Wv[rows].T).astype(NPBF16),
                "woT": woT,
                "mtri": mtri,
                "ident": ident,
            }
        )
    return in_maps


def _assemble(results):
    full = np.zeros((TF, C), dtype=np.float32)
    for c in range(NCORES):
        o = results[c]["out"].astype(np.float32)
        for g in range(NSEG):
            b, qcs, tps = SEGS[g]
            cpq = 512 // tps  # chunks per q-chunk
            qc = qcs[c // cpq]
            base = 2048 * b + 512 * qc + tps * (c % cpq)
            full[base : base + tps] = o[OUTOFF[g] : OUTOFF[g] + tps]
    return full.reshape(B, T, C)


def kernel(x, mask, Wq, Wk, Wv, Wo):
    del mask  # causal mask is hardcoded in the device kernel
    in_maps = _make_in_maps(
        np.asarray(x, dtype=np.float32),
        np.asarray(Wq, dtype=np.float32),
        np.asarray(Wk, dtype=np.float32),
        np.asarray(Wv, dtype=np.float32),
        np.asarray(Wo, dtype=np.float32),
    )
    nc = _get_nc()
    res = run_bass_kernel_spmd(nc, in_maps, core_ids=list(range(NCORES)))
    return _assemble(res.results)


# revision 20
# speedup vs baseline: 1.0298x; 1.0298x over previous
"""Distributed causal multi-head attention for TRN2 (8 NeuronCores).

Sharding: tensor-parallel over heads — core c owns heads {2c, 2c+1} for both
batches. QKV projections computed in transposed layout (feature on partitions,
tokens on free axis), attention computed as S.T = K @ Q.T per 128-key block
with softmax denominators obtained by augmenting V with a ones column. Five
segment-split AllToAlls re-shard from head-parallel to token-parallel as
attention progresses; each core then applies the output projection for its
4 x 128 tokens.

Differences from the v1 schedule (hardware-measured rationale):
- y is normalized on the SEND side: the eviction computes 1/den locally,
  broadcasts it across the 64 v-dim partitions (gpsimd ucode op), and the
  eviction copy becomes a multiply. The A2A ships normalized y (128 rows per
  chunk, no denominator row), and the post-collective path is load -> output
  projection only — no DVE op anywhere downstream of a collective.
- Head: the first-needed weight halves and x chunks fan out over 4 queues
  (sync/gpsimd/scalar/vector) so the first qkv matmul starts ~6us, not ~15.
- QKV for tokens 1024+ runs in 1024-token units (8 matmuls of free=1024) and
  the output projection in 1024-free units: fewer instructions, same PSUM.
- Output rows for the two 64-token segments (b1 qc0 / b1 qc3) are adjacent
  (384..448, 448..512); their post work is merged into one 128-token group
  so the tail after the last A2A is 8 matmuls + 1 copy + 1 store.
- Attention pairs are interleaved within each q-chunk and evictions drain
  only their own pending PV jobs, so the PE keeps a PV backlog across
  segment transitions (HAM re-throttles after ~3.4us-thin windows).
"""

import sys

sys.path.insert(0, "/opt/trn_rl_repo")

import numpy as np
import ml_dtypes

import concourse.bacc as bacc
import concourse.bass as bass
import concourse.mybir as mybir
import concourse.tile as tile
from concourse.bass_utils import run_bass_kernel_spmd

BF16 = mybir.dt.bfloat16
F32 = mybir.dt.float32
NPBF16 = ml_dtypes.bfloat16

B, T, C, H, D = 2, 2048, 1024, 16, 64
NCORES = 8
HPC = H // NCORES          # heads per core = 2
CP = HPC * D               # feature columns per core = 128
TF = B * T                 # flat tokens = 4096
TS = TF // NCORES          # output tokens per core = 512
# segments: (batch, qcs, tokens-per-core); b1's qc0 gets its own small A2A
# that triggers ~20us before attention ends, so the final barrier carries
# only qc3 and its post work is merged with qc0's (adjacent output rows)
SEGS = [
    (0, (0, 1), 128),
    (0, (2, 3), 128),
    (1, (1, 2), 128),
    (1, (0,), 64),
    (1, (3,), 64),
]
NSEG = len(SEGS)
OUTOFF = [0, 128, 256, 384, 448]
SEG_OF = {}
for _g, (_b, _qcs, _tps) in enumerate(SEGS):
    for _i, _qc in enumerate(_qcs):
        SEG_OF[(_b, _qc)] = (_g, _i, _tps)
NCB = C // 128             # feature blocks = 8
NQC = T // 512             # q-chunks per batch = 4
NKB = T // 128             # key blocks per batch = 16
SCALE = float(D) ** -0.5
MASKVAL = -30000.0
CH = 128                   # a2a chunk rows: 64 per head half, normalized y


def build_nc():
    nc = bacc.Bacc("TRN2", target_bir_lowering=False, num_devices=NCORES)

    xT = nc.dram_tensor("xT", [C, TF], BF16, kind="ExternalInput")
    # weights pre-packed on host to the on-chip layout [128, NCB, blockcols]
    wqT = nc.dram_tensor("wqT", [128, NCB * CP], BF16, kind="ExternalInput")
    wkT = nc.dram_tensor("wkT", [128, NCB * CP], BF16, kind="ExternalInput")
    wvT = nc.dram_tensor("wvT", [128, NCB * CP], BF16, kind="ExternalInput")
    woT = nc.dram_tensor("woT", [128, NCB * C], BF16, kind="ExternalInput")
    mtri = nc.dram_tensor("mtri", [128, 128], BF16, kind="ExternalInput")
    ident = nc.dram_tensor("ident", [128, 64], BF16, kind="ExternalInput")
    out = nc.dram_tensor("out", [TS, C], BF16, kind="ExternalOutput")

    with tile.TileContext(nc) as tc:
        with (
            tc.tile_pool(name="consts", bufs=1) as consts,
            tc.tile_pool(name="xp", bufs=1) as xp,
            tc.tile_pool(name="qkv", bufs=1) as qkv,
            tc.tile_pool(name="work", bufs=1) as work,
            tc.tile_pool(name="ps", bufs=1, space="PSUM") as psp,
            tc.tile_pool(name="dram", bufs=1, space="DRAM") as dram,
        ):
            # ---- weights & constants ----
            wq_sb = consts.tile([128, NCB, CP], BF16)
            wk_sb = consts.tile([128, NCB, CP], BF16)
            wv_sb = consts.tile([128, NCB, CP], BF16)
            wo_sb = consts.tile([128, NCB, C], BF16)
            mtri_sb = consts.tile([128, 128], BF16)
            ident_sb = consts.tile([128, 64], BF16)
            x_sb = [xp.tile([128, TF], BF16, name=f"x_sb{cb}") for cb in range(NCB)]
            hw_ = NCB // 2

            def wdma(eng, w_t, w_d, lo, hi):
                eng.dma_start(w_t[:, lo:hi, :], w_d[:, lo * CP : hi * CP])

            def xdma(eng, cb, t0, t1):
                eng.dma_start(x_sb[cb][:, t0:t1], xT[128 * cb : 128 * cb + 128, t0:t1])

            # Only sync/scalar (HWDGE) and gpsimd issue DMAs. Per-ring
            # transfers serialize at ~50GB/s, so chunks are placed by
            # need-time: wave0 (tokens 0-511) + wq/wk feed the prologue,
            # wave1 (512-1023) the q1 units (~15us), wave2 (1024-2047) the
            # q2 units (~28us), wave3a/b (b1 halves) the q4/q6 units
            # (~75/110us), wo the tail (~190us). The sync ring carries only
            # ~0.9MB so eviction DMAs from ~30us are never queued behind bulk.
            _q3 = (nc.sync, nc.gpsimd, nc.scalar)
            nc.scalar.dma_start(mtri_sb[:], mtri[:])
            nc.gpsimd.dma_start(ident_sb[:], ident[:])
            wdma(nc.sync, wq_sb, wqT, 0, hw_)
            wdma(nc.gpsimd, wq_sb, wqT, hw_, NCB)
            wdma(nc.scalar, wk_sb, wkT, 0, hw_)
            for cb in (0, 1, 2, 3, 4, 5):
                xdma(_q3[cb % 3], cb, 0, 512)
            wdma(nc.scalar, wk_sb, wkT, hw_, NCB)
            xdma(nc.sync, 6, 0, 512)
            xdma(nc.gpsimd, 7, 0, 512)
            # wv right after wave0: the prologue v-proj needs it ~11us
            wdma(nc.sync, wv_sb, wvT, 0, hw_)
            wdma(nc.gpsimd, wv_sb, wvT, hw_, NCB)
            # Wave 1: x tokens 512-1023
            for cb in range(NCB):
                xdma(_q3[cb % 3], cb, 512, 1024)
            # Waves 2a / 3a / 2b / 3b interleaved by feed consumption order
            # (q2 ~26us, q4/q5 ~32us, q3 ~40us, q6/q7 ~55us): while q3's
            # tranche is still in flight the PE chews the b1 first-half qkv
            for cb in range(NCB):
                xdma(_q3[cb % 3], cb, 1024, 1536)
            for cb in range(NCB):
                xdma(_q3[(cb + 1) % 3], cb, 2048, 3072)
            for cb in range(NCB):
                xdma(_q3[(cb + 2) % 3], cb, 1536, 2048)
            for cb in range(NCB):
                xdma(_q3[cb % 3], cb, 3072, 4096)
            # wo rides the scalar ring (its issues all precede the first exp,
            # and its transfers contend with nothing the PE waits on) so the
            # gpsimd ring is clear for the post y loads by ~85us
            nc.scalar.dma_start(wo_sb[:], woT[:])

            qT_sb = qkv.tile([128, TF], BF16)
            kT_sb = qkv.tile([128, TF], BF16)
            vT_sb = qkv.tile([128, TF], BF16)
            projs = ((wq_sb, qT_sb), (wk_sb, kT_sb), (wv_sb, vT_sb))

            v_sb = [work.tile([128, NKB, 65], BF16, name=f"v_sb{p}") for p in range(4)]

            # A2A groups: segs 2+3 share one collective (their evictions
            # complete back-to-back, and a separate seg3 A2A would serialize
            # behind seg2's on the CC stream, adding a full ~15us flight)
            GRP_W = [128, 128, 192, 64]        # chunk cols per group
            SEG_GRP = {0: (0, 0), 1: (1, 0), 2: (2, 0), 3: (2, 128), 4: (3, 0)}
            a2a_in = [
                dram.tile([NCORES * CH, GRP_W[g]], BF16, name=f"a2a_in{g}")
                for g in range(4)
            ]
            a2a_out = [
                dram.tile([NCORES * CH, GRP_W[g]], BF16, name=f"a2a_out{g}")
                for g in range(4)
            ]

            # ---------- emission units ----------
            def qkv_unit(tcn, pi):
                # 512-token units: each gates on exactly one x tranche DMA
                w_sb, oT = projs[pi]
                t0 = 512 * tcn
                ps = psp.tile(
                    [128, 512], F32, tag="st", bufs=3,
                    padded_shape=[128, 1024], name="ps_proj",
                )
                for cb in range(NCB):
                    nc.tensor.matmul(
                        ps[:],
                        lhsT=w_sb[:, cb, :],
                        rhs=x_sb[cb][:, t0 : t0 + 512],
                        start=(cb == 0),
                        stop=(cb == NCB - 1),
                    )
                if pi == 1:
                    nc.scalar.copy(oT[:, t0 : t0 + 512], ps[:])
                else:
                    nc.vector.tensor_copy(oT[:, t0 : t0 + 512], ps[:])

            def vt_unit(pair, kb):
                hh, b = pair % 2, pair // 2
                if kb == 0:
                    nc.vector.memset(v_sb[pair][:, :, 64:65], 1.0)
                t0 = 2048 * b + 128 * kb
                vt_ps = psp.tile([128, 64], BF16, tag="ot", bufs=2, name="vt_ps")
                nc.tensor.transpose(
                    vt_ps[:],
                    vT_sb[64 * hh : 64 * hh + 64, t0 : t0 + 128],
                    ident_sb[64 * hh : 64 * hh + 64, :],
                )
                nc.vector.tensor_copy(v_sb[pair][:, kb, 0:64], vt_ps[:])

            # attention state per (pair, qc), lives across kbp units
            attn_ot = {}
            pending_pv = []

            def emit_pv(job):
                pair, qc, kbp, pT, offs = job
                ot = attn_ot[(pair, qc)]
                n_kb = 4 * qc + 4
                for h2 in range(2):
                    kb = 2 * kbp + h2
                    off = offs[h2]
                    nc.tensor.matmul(
                        ot[:, off:512],
                        lhsT=v_sb[pair][:, kb, :],
                        rhs=pT[:, 512 * h2 + off : 512 * h2 + 512],
                        start=(kb == 0),
                        stop=(kb == n_kb - 1),
                    )

            def drain_pending(pair=None, qc=None):
                rest = []
                for job in pending_pv:
                    if pair is None or (job[0] == pair and job[1] == qc):
                        emit_pv(job)
                    else:
                        rest.append(job)
                pending_pv[:] = rest

            def attn_unit(pair, qc, kbp):
                hh, b = pair % 2, pair // 2
                hs = slice(64 * hh, 64 * hh + 64)
                tb0 = 2048 * b
                q0 = tb0 + 512 * qc
                if kbp == 0:
                    attn_ot[(pair, qc)] = psp.tile(
                        [65, 512], F32, tag="ot", bufs=2, name="ot_ps"
                    )
                st = psp.tile([128, 1024], F32, tag="st", bufs=3, name="st_ps")
                offs = []
                for h2 in range(2):
                    kb = 2 * kbp + h2
                    off = max(0, 128 * kb - 512 * qc)
                    offs.append(off)
                    nc.tensor.matmul(
                        st[:, 512 * h2 + off : 512 * h2 + 512],
                        lhsT=kT_sb[hs, tb0 + 128 * kb : tb0 + 128 * kb + 128],
                        rhs=qT_sb[hs, q0 + off : q0 + 512],
                        start=True,
                        stop=True,
                    )
                for h2 in range(2):
                    kb = 2 * kbp + h2
                    if 128 * kb >= 512 * qc:  # diagonal block: triangular mask
                        off = offs[h2]
                        dd = slice(512 * h2 + off, 512 * h2 + off + 128)
                        nc.vector.tensor_add(st[:, dd], st[:, dd], mtri_sb[:])
                pT = work.tile([128, 1024], BF16, tag="pT", bufs=8, name="pT")
                o0 = offs[0]
                nc.scalar.activation(
                    pT[:, o0:1024],
                    st[:, o0:1024],
                    mybir.ActivationFunctionType.Exp,
                    scale=SCALE,
                )
                pending_pv.append((pair, qc, kbp, pT, offs))
                if len(pending_pv) > 3:
                    emit_pv(pending_pv.pop(0))

            def evict_unit(pair, qc):
                drain_pending(pair, qc)
                hh, b = pair % 2, pair // 2
                g, qi, tps = SEG_OF[(b, qc)]
                nch = 512 // tps          # chunks this eviction covers
                s0 = nch * qi
                ot = attn_ot.pop((pair, qc))
                # send-side normalization: reciprocal of the denominator row,
                # broadcast across the 64 v-dims, folded into the eviction
                # copy. All inputs are local (PSUM), so nothing anywhere is
                # gated on a collective, and the post-A2A path needs no DVE.
                den32 = work.tile([1, 512], F32, tag="den32", bufs=3, name="den32")
                rec32 = work.tile([1, 512], F32, tag="rec32", bufs=3, name="rec32")
                recbc = work.tile([64, 512], F32, tag="recbc", bufs=2, name="recbc")
                y_sb = work.tile([64, 512], BF16, tag="y", bufs=12, name="y_sb")
                # stage through SBUF: the approx's bit-level seed needs raw
                # IEEE fp32, so don't feed it PSUM directly
                nc.vector.tensor_copy(den32[:], ot[64:65, :])
                nc.vector.reciprocal_approx_fast(rec32[:], den32[:])
                nc.gpsimd.partition_broadcast(recbc[:], rec32[:], channels=64)
                nc.vector.tensor_mul(y_sb[:], ot[0:64, :], recbc[:])
                grp, coff = SEG_GRP[g]
                w = GRP_W[grp]
                ydst = bass.AP(
                    a2a_in[grp].tensor,
                    (s0 * CH + 64 * hh) * w + coff,
                    [[w, 64], [CH * w, nch], [1, tps]],
                )
                nc.sync.dma_start(ydst, y_sb[:, :])

            def coll_unit(grp):
                nc.gpsimd.collective_compute(
                    "AllToAll",
                    mybir.AluOpType.bypass,
                    replica_groups=[list(range(NCORES))],
                    ins=[a2a_in[grp][:].opt()],
                    outs=[a2a_out[grp][:].opt()],
                )

            # post y tiles: segs 0-2 individually; segs 3+4 share one tile
            # (adjacent 64-token column halves -> one 128-token proj group)
            y_locs = {}

            def post_dma(g):
                # gpsimd queue: these loads gate on the seg's A2A completing
                tps = SEGS[g][2]
                grp, coff = SEG_GRP[g]
                w = GRP_W[grp]
                if g < 3:
                    y_loc = work.tile(
                        [128, NCB, tps], BF16, tag="yloc", bufs=3, name="y_loc"
                    )
                    dst0 = 0
                    y_locs[g] = y_loc
                else:
                    if 3 not in y_locs:
                        y_locs[3] = work.tile(
                            [128, NCB, 128], BF16, tag="yloc34", bufs=1, name="y_loc34"
                        )
                    y_loc = y_locs[3]
                    dst0 = 64 * (g - 3)
                ysrc = bass.AP(
                    a2a_out[grp].tensor,
                    coff,
                    [[w, 128], [CH * w, NCB], [1, tps]],
                )
                nc.gpsimd.dma_start(y_loc[:, :, dst0 : dst0 + tps], ysrc)

            def proj_group(g):
                # g in {0,1,2}: 128 tokens at OUTOFF[g]; g==3: merged segs 3+4
                y_loc = y_locs[g]
                r0 = OUTOFF[g] if g < 3 else OUTOFF[3]
                ps = psp.tile(
                    [128, C], F32, tag="st", bufs=3,
                    padded_shape=[128, 1024], name="ps_op",
                )
                for mh in range(2):  # matmul out is one PSUM bank
                    for cb in range(NCB):
                        nc.tensor.matmul(
                            ps[:, 512 * mh : 512 * mh + 512],
                            lhsT=y_loc[:, cb, :],
                            rhs=wo_sb[:, cb, 512 * mh : 512 * mh + 512],
                            start=(cb == 0),
                            stop=(cb == NCB - 1),
                        )
                o_sb = work.tile([128, C], BF16, tag="osb", bufs=2, name="o_sb")
                nc.vector.tensor_copy(o_sb[:], ps[:])
                nc.sync.dma_start(out[r0 : r0 + 128, :], o_sb[:])

            def attn_units_for_seg(g):
                b, qcs, _ = SEGS[g]
                units = []
                for qc in qcs:
                    for hh in range(2):
                        pair = 2 * b + hh
                        for kbp in range(2 * qc + 2):
                            units.append(("a", pair, qc, kbp))
                        units.append(("e", pair, qc))
                return units

            def run_unit(u):
                if u[0] == "a":
                    attn_unit(u[1], u[2], u[3])
                elif u[0] == "e":
                    evict_unit(u[1], u[2])
                elif u[0] == "q":
                    qkv_unit(u[1], u[2])
                elif u[0] == "v":
                    vt_unit(u[1], u[2])

            feed = []
            fi = 0

            def run_chunk(main):
                nonlocal fi
                for u in main:
                    run_unit(u)
                    if fi < len(feed):
                        run_unit(feed[fi])
                        fi += 1

            def run_feed_one():
                nonlocal fi
                if fi < len(feed):
                    run_unit(feed[fi])
                    fi += 1

            def drain_feed():
                nonlocal fi
                while fi < len(feed):
                    run_unit(feed[fi])
                    fi += 1

            # ---------- schedule ----------
            # minimal prologue: only what attention (b0, qc0) needs up-front
            for pi in range(3):
                qkv_unit(0, pi)
            for hh in range(2):
                for kb in range(4):
                    vt_unit(hh, kb)

            feed += [("q", 1, pi) for pi in range(3)]
            feed += [("v", hh, kb) for hh in range(2) for kb in range(4, 8)]
            feed += [("q", 2, pi) for pi in range(3)]
            feed += [("v", 0, kb) for kb in range(8, 12)]
            feed += [("q", 4, pi) for pi in range(3)]
            feed += [("q", 3, pi) for pi in range(3)]
            feed += [("v", 1, kb) for kb in range(8, 12)]
            feed += [("v", 0, kb) for kb in range(12, NKB)]
            feed += [("q", 5, pi) for pi in range(3)]
            feed += [("v", 1, kb) for kb in range(12, NKB)]
            feed += [("v", 2 + hh, kb) for hh in range(2) for kb in range(8)]
            feed += [("q", 6, pi) for pi in range(3)]
            feed += [("q", 7, pi) for pi in range(3)]
            feed += [("v", 2 + hh, kb) for hh in range(2) for kb in range(8, 12)]
            # b1 v transposes for kb 12-15 are only needed by qc3 — they are
            # held back as PE filler for the thin b1-qc0 stretch below
            late_vt = [("v", 2 + hh, kb) for hh in range(2) for kb in range(12, NKB)]

            run_chunk(attn_units_for_seg(0))              # b0 qc0-1
            coll_unit(0)
            run_chunk(attn_units_for_seg(1))              # b0 qc2-3
            coll_unit(1)
            drain_feed()  # all b1 qkv/vt must be emitted before b1 attention
            main2 = attn_units_for_seg(2)                 # b1 qc1-2
            for i, u in enumerate(main2):
                if i == 20:
                    post_dma(0)
                run_unit(u)
            main3 = attn_units_for_seg(3)                 # b1 qc0 (6 units)
            vt_i = 0
            for i, u in enumerate(main3):
                if i == 2:
                    post_dma(1)
                run_unit(u)
                for _ in range(2):
                    if vt_i < len(late_vt):
                        run_unit(late_vt[vt_i])
                        vt_i += 1
                if i == 3:
                    # seg0's projection (inputs landed ~60us ago, nothing
                    # collective-gated) fills this thin stretch so HAM does
                    # not re-throttle right before qc3's attention
                    proj_group(0)
            coll_unit(2)  # merged segs 2+3 exchange
            main4 = attn_units_for_seg(4)                 # b1 qc3 (18 units)
            for i, u in enumerate(main4):
                if i == 2:
                    post_dma(2)
                if i == 4:
                    post_dma(3)
                run_unit(u)
            coll_unit(3)
            post_dma(4)
            # tail: held-back projections fill the final collective's flight
            # time; the seg3 half of the merged 3+4 group runs as soon as the
            # merged A2A lands, leaving only the seg4 half + copy + store
            # gated on the final collective
            proj_group(1)
            proj_group(2)
            y34 = y_locs[3]
            ps34 = psp.tile(
                [128, C], F32, tag="st", bufs=3,
                padded_shape=[128, 1024], name="ps34",
            )
            for half in range(2):  # 0: seg3 (gated merged A2A), 1: seg4
                for mh in range(2):
                    for cb in range(NCB):
                        nc.tensor.matmul(
                            ps34[64 * half : 64 * half + 64,
                                 512 * mh : 512 * mh + 512],
                            lhsT=y34[:, cb, 64 * half : 64 * half + 64],
                            rhs=wo_sb[:, cb, 512 * mh : 512 * mh + 512],
                            start=(cb == 0),
                            stop=(cb == NCB - 1),
                        )
            o34 = work.tile([128, C], BF16, tag="osb", bufs=2, name="o34")
            nc.vector.tensor_copy(o34[:], ps34[:])
            nc.sync.dma_start(out[OUTOFF[3] : OUTOFF[3] + 128, :], o34[:])

    nc.compile()
    return nc


_NC = None


def _get_nc():
    global _NC
    if _NC is None:
        _NC = build_nc()
    return _NC


def _host_consts():
    idx = np.arange(128)
    mtri = np.where(idx[None, :] >= idx[:, None], 0.0, MASKVAL).astype(np.float32)
    ident = np.concatenate([np.eye(64, dtype=np.float32)] * 2, axis=0)
    return mtri.astype(NPBF16), ident.astype(NPBF16)


def _pack_w(wT):
    # [C, cols] -> [128, NCB*cols]: w_sb[p, cb*cols+j] = wT[128*cb+p, j]
    cols = wT.shape[1]
    return np.ascontiguousarray(
        wT.reshape(NCB, 128, cols).transpose(1, 0, 2).reshape(128, NCB * cols)
    )


def _make_in_maps(x, Wq, Wk, Wv, Wo):
    xT = np.ascontiguousarray(x.reshape(TF, C).T).astype(NPBF16)
    woT = _pack_w(Wo.T).astype(NPBF16)
    mtri, ident = _host_consts()
    in_maps = []
    for c in range(NCORES):
        rows = slice(CP * c, CP * c + CP)
        in_maps.append(
            {
                "xT": xT,
                "wqT": _pack_w(Wq[rows].T).astype(NPBF16),
                "wkT": _pack_w(Wk[rows].T).astype(NPBF16),
                "wvT": _pack_w(Wv[rows].T).astype(NPBF16),
                "woT": woT,
                "mtri": mtri,
                "ident": ident,
            }
        )
    return in_maps


def _assemble(results):
    full = np.zeros((TF, C), dtype=np.float32)
    for c in range(NCORES):
        o = results[c]["out"].astype(np.float32)
        for g in range(NSEG):
            b, qcs, tps = SEGS[g]
            cpq = 512 // tps  # chunks per q-chunk
            qc = qcs[c // cpq]
            base = 2048 * b + 512 * qc + tps * (c % cpq)
            full[base : base + tps] = o[OUTOFF[g] : OUTOFF[g] + tps]
    return full.reshape(B, T, C)


def kernel(x, mask, Wq, Wk, Wv, Wo):
    del mask  # causal mask is hardcoded in the device kernel
    in_maps = _make_in_maps(
        np.asarray(x, dtype=np.float32),
        np.asarray(Wq, dtype=np.float32),
        np.asarray(Wk, dtype=np.float32),
        np.asarray(Wv, dtype=np.float32),
        np.asarray(Wo, dtype=np.float32),
    )
    nc = _get_nc()
    res = run_bass_kernel_spmd(nc, in_maps, core_ids=list(range(NCORES)))
    return _assemble(res.results)


# revision 24
# speedup vs baseline: 1.0470x; 1.0167x over previous
"""Distributed causal multi-head attention for TRN2 (8 NeuronCores).

Sharding: tensor-parallel over heads — core c owns heads {2c, 2c+1} for both
batches. QKV projections computed in transposed layout (feature on partitions,
tokens on free axis), attention computed as S.T = K @ Q.T per 128-key block
with softmax denominators obtained by augmenting V with a ones column. Four
AllToAlls re-shard from head-parallel to token-parallel as attention
progresses; each core then applies the output projection for its 512 tokens.

Scheduling notes (hardware-measured):
- y is normalized on the SEND side: the eviction computes 1/den locally,
  broadcasts it across the 64 v-dim partitions with a stride-0 SBUF->SBUF
  DMA on sync, and the eviction copy becomes a multiply. The A2A ships
  normalized y (128 rows per chunk, no denominator row), so the post path
  is load -> projection with no DVE op downstream of any collective.
  The broadcast must NOT ride the gpsimd queue: collective_compute blocks
  that queue for the full trigger..completion flight (~15-30us), and an
  eviction ingredient parked there stalls the PE via the PSUM ot pool.
- Segs 2+3 (b1 qc1-2 and qc0) share one A2A: their evictions finish
  back-to-back and separate collectives serialize on the CC stream at a
  full flight (~15us) each.
- The two 64-token segments (b1 qc0 / qc3) write adjacent output rows
  (384..448..512); their projections share one PSUM tile, with the qc0
  half gated on the merged A2A so only the qc3 half + copy + store trail
  the final collective.
- x/weights land by need-time on the 3 DMA rings (sync/gpsimd/scalar,
  each ~50GB/s, ~FIFO): wave0+wq/wk -> prologue, wv -> v-proj (~11us),
  wave1 -> q1 (~18us), wave2a -> q2 (~26us), wave3a -> q4/q5 (~32us),
  wave2b -> q3 (~40us), wave3b -> q6/q7 (~55us), wo (tail) last on scalar.
  While q3's tranche is in flight the PE chews b1's first-half qkv.
- seg0's output projection runs inside the thin b1-qc0 stretch (its
  inputs landed ~60us earlier) so HAM does not re-throttle before qc3;
  the b1 kb12-15 v transposes are held back for the same purpose.
- No warmup collective: the CC path has an absolute ~74us cold-start, so
  a warmup A2A only pushes the real A2A#0 later (measured +11us).
"""

import sys

sys.path.insert(0, "/opt/trn_rl_repo")

import numpy as np
import ml_dtypes

import concourse.bacc as bacc
import concourse.bass as bass
import concourse.mybir as mybir
import concourse.tile as tile
from concourse.bass_utils import run_bass_kernel_spmd

BF16 = mybir.dt.bfloat16
F32 = mybir.dt.float32
NPBF16 = ml_dtypes.bfloat16

B, T, C, H, D = 2, 2048, 1024, 16, 64
NCORES = 8
HPC = H // NCORES          # heads per core = 2
CP = HPC * D               # feature columns per core = 128
TF = B * T                 # flat tokens = 4096
TS = TF // NCORES          # output tokens per core = 512
# segments: (batch, qcs, tokens-per-core); b1's qc0 gets its own small A2A
# that triggers ~20us before attention ends, so the final barrier carries
# only qc3 and its post work is merged with qc0's (adjacent output rows)
SEGS = [
    (0, (0, 1), 128),
    (0, (2, 3), 128),
    (1, (1, 2), 128),
    (1, (0,), 64),
    (1, (3,), 64),
]
NSEG = len(SEGS)
OUTOFF = [0, 128, 256, 384, 448]
SEG_OF = {}
for _g, (_b, _qcs, _tps) in enumerate(SEGS):
    for _i, _qc in enumerate(_qcs):
        SEG_OF[(_b, _qc)] = (_g, _i, _tps)
NCB = C // 128             # feature blocks = 8
NQC = T // 512             # q-chunks per batch = 4
NKB = T // 128             # key blocks per batch = 16
SCALE = float(D) ** -0.5
MASKVAL = -30000.0
CH = 128                   # a2a chunk rows: 64 per head half, normalized y


def build_nc():
    nc = bacc.Bacc("TRN2", target_bir_lowering=False, num_devices=NCORES)

    xT = nc.dram_tensor("xT", [C, TF], BF16, kind="ExternalInput")
    # weights pre-packed on host to the on-chip layout [128, NCB, blockcols]
    wqT = nc.dram_tensor("wqT", [128, NCB * CP], BF16, kind="ExternalInput")
    wkT = nc.dram_tensor("wkT", [128, NCB * CP], BF16, kind="ExternalInput")
    wvT = nc.dram_tensor("wvT", [128, NCB * CP], BF16, kind="ExternalInput")
    woT = nc.dram_tensor("woT", [128, NCB * C], BF16, kind="ExternalInput")
    mtri = nc.dram_tensor("mtri", [128, 128], BF16, kind="ExternalInput")
    ident = nc.dram_tensor("ident", [128, 64], BF16, kind="ExternalInput")
    out = nc.dram_tensor("out", [TS, C], BF16, kind="ExternalOutput")

    with tile.TileContext(nc) as tc:
        with (
            tc.tile_pool(name="consts", bufs=1) as consts,
            tc.tile_pool(name="xp", bufs=1) as xp,
            tc.tile_pool(name="qkv", bufs=1) as qkv,
            tc.tile_pool(name="work", bufs=1) as work,
            tc.tile_pool(name="ps", bufs=1, space="PSUM") as psp,
            tc.tile_pool(name="dram", bufs=1, space="DRAM") as dram,
        ):
            # ---- weights & constants ----
            wq_sb = consts.tile([128, NCB, CP], BF16)
            wk_sb = consts.tile([128, NCB, CP], BF16)
            wv_sb = consts.tile([128, NCB, CP], BF16)
            wo_sb = consts.tile([128, NCB, C], BF16)
            mtri_sb = consts.tile([128, 128], BF16)
            ident_sb = consts.tile([128, 64], BF16)
            x_sb = [xp.tile([128, TF], BF16, name=f"x_sb{cb}") for cb in range(NCB)]
            hw_ = NCB // 2

            def wdma(eng, w_t, w_d, lo, hi):
                eng.dma_start(w_t[:, lo:hi, :], w_d[:, lo * CP : hi * CP])

            def xdma(eng, cb, t0, t1):
                eng.dma_start(x_sb[cb][:, t0:t1], xT[128 * cb : 128 * cb + 128, t0:t1])

            # Only sync/scalar (HWDGE) and gpsimd issue DMAs. Per-ring
            # transfers serialize at ~50GB/s, so chunks are placed by
            # need-time: wave0 (tokens 0-511) + wq/wk feed the prologue,
            # wave1 (512-1023) the q1 units (~15us), wave2 (1024-2047) the
            # q2 units (~28us), wave3a/b (b1 halves) the q4/q6 units
            # (~75/110us), wo the tail (~190us). The sync ring carries only
            # ~0.9MB so eviction DMAs from ~30us are never queued behind bulk.
            _q3 = (nc.sync, nc.gpsimd, nc.scalar)
            nc.scalar.dma_start(mtri_sb[:], mtri[:])
            nc.gpsimd.dma_start(ident_sb[:], ident[:])
            wdma(nc.sync, wq_sb, wqT, 0, hw_)
            wdma(nc.gpsimd, wq_sb, wqT, hw_, NCB)
            wdma(nc.scalar, wk_sb, wkT, 0, hw_)
            for cb in (0, 1, 2, 3, 4, 5):
                xdma(_q3[cb % 3], cb, 0, 512)
            wdma(nc.scalar, wk_sb, wkT, hw_, NCB)
            xdma(nc.sync, 6, 0, 512)
            xdma(nc.gpsimd, 7, 0, 512)
            # wv right after wave0: the prologue v-proj needs it ~11us
            wdma(nc.sync, wv_sb, wvT, 0, hw_)
            wdma(nc.gpsimd, wv_sb, wvT, hw_, NCB)
            # Wave 1: x tokens 512-1023
            for cb in range(NCB):
                xdma(_q3[cb % 3], cb, 512, 1024)
            # Waves 2a / 3a / 2b / 3b interleaved by feed consumption order
            # (q2 ~26us, q4/q5 ~32us, q3 ~40us, q6/q7 ~55us): while q3's
            # tranche is still in flight the PE chews the b1 first-half qkv
            for cb in range(NCB):
                xdma(_q3[cb % 3], cb, 1024, 1536)
            for cb in range(NCB):
                xdma(_q3[(cb + 1) % 3], cb, 2048, 3072)
            for cb in range(NCB):
                xdma(_q3[(cb + 2) % 3], cb, 1536, 2048)
            for cb in range(NCB):
                xdma(_q3[cb % 3], cb, 3072, 4096)
            # wo rides the scalar ring (its issues all precede the first exp,
            # and its transfers contend with nothing the PE waits on) so the
            # gpsimd ring is clear for the post y loads by ~85us
            nc.scalar.dma_start(wo_sb[:], woT[:])

            qT_sb = qkv.tile([128, TF], BF16)
            kT_sb = qkv.tile([128, TF], BF16)
            vT_sb = qkv.tile([128, TF], BF16)
            projs = ((wq_sb, qT_sb), (wk_sb, kT_sb), (wv_sb, vT_sb))

            v_sb = [work.tile([128, NKB, 65], BF16, name=f"v_sb{p}") for p in range(4)]

            # A2A groups: segs 2+3 share one collective (their evictions
            # complete back-to-back, and a separate seg3 A2A would serialize
            # behind seg2's on the CC stream, adding a full ~15us flight)
            GRP_W = [128, 128, 192, 64]        # chunk cols per group
            SEG_GRP = {0: (0, 0), 1: (1, 0), 2: (2, 0), 3: (2, 128), 4: (3, 0)}
            a2a_in = [
                dram.tile([NCORES * CH, GRP_W[g]], BF16, name=f"a2a_in{g}")
                for g in range(4)
            ]
            a2a_out = [
                dram.tile([NCORES * CH, GRP_W[g]], BF16, name=f"a2a_out{g}")
                for g in range(4)
            ]

            # ---------- emission units ----------
            def qkv_unit(tcn, pi):
                # 512-token units: each gates on exactly one x tranche DMA
                w_sb, oT = projs[pi]
                t0 = 512 * tcn
                ps = psp.tile(
                    [128, 512], F32, tag="st", bufs=3,
                    padded_shape=[128, 1024], name="ps_proj",
                )
                for cb in range(NCB):
                    nc.tensor.matmul(
                        ps[:],
                        lhsT=w_sb[:, cb, :],
                        rhs=x_sb[cb][:, t0 : t0 + 512],
                        start=(cb == 0),
                        stop=(cb == NCB - 1),
                    )
                if pi == 1:
                    nc.scalar.copy(oT[:, t0 : t0 + 512], ps[:])
                else:
                    nc.vector.tensor_copy(oT[:, t0 : t0 + 512], ps[:])

            def vt_unit(pair, kb):
                hh, b = pair % 2, pair // 2
                if kb == 0:
                    nc.vector.memset(v_sb[pair][:, :, 64:65], 1.0)
                t0 = 2048 * b + 128 * kb
                vt_ps = psp.tile([128, 64], BF16, tag="ot", bufs=2, name="vt_ps")
                nc.tensor.transpose(
                    vt_ps[:],
                    vT_sb[64 * hh : 64 * hh + 64, t0 : t0 + 128],
                    ident_sb[64 * hh : 64 * hh + 64, :],
                )
                nc.vector.tensor_copy(v_sb[pair][:, kb, 0:64], vt_ps[:])

            # attention state per (pair, qc), lives across kbp units
            attn_ot = {}
            pending_pv = []

            def emit_pv(job):
                pair, qc, kbp, pT, offs = job
                ot = attn_ot[(pair, qc)]
                n_kb = 4 * qc + 4
                for h2 in range(2):
                    kb = 2 * kbp + h2
                    off = offs[h2]
                    nc.tensor.matmul(
                        ot[:, off:512],
                        lhsT=v_sb[pair][:, kb, :],
                        rhs=pT[:, 512 * h2 + off : 512 * h2 + 512],
                        start=(kb == 0),
                        stop=(kb == n_kb - 1),
                    )

            def drain_pending(pair=None, qc=None):
                rest = []
                for job in pending_pv:
                    if pair is None or (job[0] == pair and job[1] == qc):
                        emit_pv(job)
                    else:
                        rest.append(job)
                pending_pv[:] = rest

            def attn_unit(pair, qc, kbp):
                hh, b = pair % 2, pair // 2
                hs = slice(64 * hh, 64 * hh + 64)
                tb0 = 2048 * b
                q0 = tb0 + 512 * qc
                if kbp == 0:
                    attn_ot[(pair, qc)] = psp.tile(
                        [65, 512], F32, tag="ot", bufs=2, name="ot_ps"
                    )
                st = psp.tile([128, 1024], F32, tag="st", bufs=3, name="st_ps")
                offs = []
                for h2 in range(2):
                    kb = 2 * kbp + h2
                    off = max(0, 128 * kb - 512 * qc)
                    offs.append(off)
                    nc.tensor.matmul(
                        st[:, 512 * h2 + off : 512 * h2 + 512],
                        lhsT=kT_sb[hs, tb0 + 128 * kb : tb0 + 128 * kb + 128],
                        rhs=qT_sb[hs, q0 + off : q0 + 512],
                        start=True,
                        stop=True,
                    )
                for h2 in range(2):
                    kb = 2 * kbp + h2
                    if 128 * kb >= 512 * qc:  # diagonal block: triangular mask
                        off = offs[h2]
                        dd = slice(512 * h2 + off, 512 * h2 + off + 128)
                        nc.vector.tensor_add(st[:, dd], st[:, dd], mtri_sb[:])
                pT = work.tile([128, 1024], BF16, tag="pT", bufs=8, name="pT")
                o0 = offs[0]
                nc.scalar.activation(
                    pT[:, o0:1024],
                    st[:, o0:1024],
                    mybir.ActivationFunctionType.Exp,
                    scale=SCALE,
                )
                pending_pv.append((pair, qc, kbp, pT, offs))
                if len(pending_pv) > 3:
                    emit_pv(pending_pv.pop(0))

            def evict_unit(pair, qc):
                drain_pending(pair, qc)
                hh, b = pair % 2, pair // 2
                g, qi, tps = SEG_OF[(b, qc)]
                nch = 512 // tps          # chunks this eviction covers
                s0 = nch * qi
                ot = attn_ot.pop((pair, qc))
                # send-side normalization: reciprocal of the denominator row,
                # broadcast across the 64 v-dims, folded into the eviction
                # copy. All inputs are local (PSUM), so nothing anywhere is
                # gated on a collective, and the post-A2A path needs no DVE.
                den32 = work.tile([1, 512], F32, tag="den32", bufs=3, name="den32")
                rec32 = work.tile([1, 512], F32, tag="rec32", bufs=3, name="rec32")
                rec_dr = dram.tile([1, 512], F32, tag="recdr", bufs=3, name="rec_dr")
                recbc = work.tile([64, 512], F32, tag="recbc", bufs=2, name="recbc")
                y_sb = work.tile([64, 512], BF16, tag="y", bufs=12, name="y_sb")
                # stage through SBUF: the approx's bit-level seed needs raw
                # IEEE fp32, so don't feed it PSUM directly
                nc.vector.tensor_copy(den32[:], ot[64:65, :])
                nc.vector.reciprocal_approx_fast(rec32[:], den32[:])
                # broadcast 1/den across the 64 v-dim partitions via a tiny
                # DRAM bounce with a stride-0 source AP, both DMAs on sync:
                # the gpsimd queue blocks for each collective's full flight
                # (trigger..completion), and an eviction ingredient parked
                # there stalls the PE via the PSUM ot pool for tens of us.
                # (SBUF sources reject stride-0 partition dims; DRAM is fine.)
                nc.sync.dma_start(rec_dr[:], rec32[:])
                _r = rec_dr[:]
                rec_bc_src = bass.AP(_r.tensor, _r.offset, [[0, 64], [1, 512]])
                nc.sync.dma_start(recbc[:], rec_bc_src)
                nc.vector.tensor_mul(y_sb[:], ot[0:64, :], recbc[:])
                grp, coff = SEG_GRP[g]
                w = GRP_W[grp]
                ydst = bass.AP(
                    a2a_in[grp].tensor,
                    (s0 * CH + 64 * hh) * w + coff,
                    [[w, 64], [CH * w, nch], [1, tps]],
                )
                nc.sync.dma_start(ydst, y_sb[:, :])

            def coll_unit(grp):
                nc.gpsimd.collective_compute(
                    "AllToAll",
                    mybir.AluOpType.bypass,
                    replica_groups=[list(range(NCORES))],
                    ins=[a2a_in[grp][:].opt()],
                    outs=[a2a_out[grp][:].opt()],
                )

            # post y tiles: segs 0-2 individually; segs 3+4 share one tile
            # (adjacent 64-token column halves -> one 128-token proj group)
            y_locs = {}

            def post_dma(g):
                # gpsimd queue: these loads gate on the seg's A2A completing
                tps = SEGS[g][2]
                grp, coff = SEG_GRP[g]
                w = GRP_W[grp]
                if g < 3:
                    y_loc = work.tile(
                        [128, NCB, tps], BF16, tag="yloc", bufs=3, name="y_loc"
                    )
                    dst0 = 0
                    y_locs[g] = y_loc
                else:
                    if 3 not in y_locs:
                        y_locs[3] = work.tile(
                            [128, NCB, 128], BF16, tag="yloc34", bufs=1, name="y_loc34"
                        )
                    y_loc = y_locs[3]
                    dst0 = 64 * (g - 3)
                ysrc = bass.AP(
                    a2a_out[grp].tensor,
                    coff,
                    [[w, 128], [CH * w, NCB], [1, tps]],
                )
                nc.gpsimd.dma_start(y_loc[:, :, dst0 : dst0 + tps], ysrc)

            def proj_group(g):
                # g in {0,1,2}: 128 tokens at OUTOFF[g]; g==3: merged segs 3+4
                y_loc = y_locs[g]
                r0 = OUTOFF[g] if g < 3 else OUTOFF[3]
                ps = psp.tile(
                    [128, C], F32, tag="st", bufs=3,
                    padded_shape=[128, 1024], name="ps_op",
                )
                for mh in range(2):  # matmul out is one PSUM bank
                    for cb in range(NCB):
                        nc.tensor.matmul(
                            ps[:, 512 * mh : 512 * mh + 512],
                            lhsT=y_loc[:, cb, :],
                            rhs=wo_sb[:, cb, 512 * mh : 512 * mh + 512],
                            start=(cb == 0),
                            stop=(cb == NCB - 1),
                        )
                o_sb = work.tile([128, C], BF16, tag="osb", bufs=2, name="o_sb")
                nc.vector.tensor_copy(o_sb[:], ps[:])
                nc.sync.dma_start(out[r0 : r0 + 128, :], o_sb[:])

            def attn_units_for_seg(g):
                b, qcs, _ = SEGS[g]
                units = []
                for qc in qcs:
                    for hh in range(2):
                        pair = 2 * b + hh
                        for kbp in range(2 * qc + 2):
                            units.append(("a", pair, qc, kbp))
                        units.append(("e", pair, qc))
                return units

            def run_unit(u):
                if u[0] == "a":
                    attn_unit(u[1], u[2], u[3])
                elif u[0] == "e":
                    evict_unit(u[1], u[2])
                elif u[0] == "q":
                    qkv_unit(u[1], u[2])
                elif u[0] == "v":
                    vt_unit(u[1], u[2])

            feed = []
            fi = 0

            def run_chunk(main):
                nonlocal fi
                for u in main:
                    run_unit(u)
                    if fi < len(feed):
                        run_unit(feed[fi])
                        fi += 1

            def run_feed_one():
                nonlocal fi
                if fi < len(feed):
                    run_unit(feed[fi])
                    fi += 1

            def drain_feed():
                nonlocal fi
                while fi < len(feed):
                    run_unit(feed[fi])
                    fi += 1

            # ---------- schedule ----------
            # minimal prologue: only what attention (b0, qc0) needs up-front
            for pi in range(3):
                qkv_unit(0, pi)
            for hh in range(2):
                for kb in range(4):
                    vt_unit(hh, kb)

            feed += [("q", 1, pi) for pi in range(3)]
            feed += [("v", hh, kb) for hh in range(2) for kb in range(4, 8)]
            feed += [("q", 2, pi) for pi in range(3)]
            feed += [("v", 0, kb) for kb in range(8, 12)]
            feed += [("q", 4, pi) for pi in range(3)]
            feed += [("q", 3, pi) for pi in range(3)]
            feed += [("v", 1, kb) for kb in range(8, 12)]
            feed += [("v", 0, kb) for kb in range(12, NKB)]
            feed += [("q", 5, pi) for pi in range(3)]
            feed += [("v", 1, kb) for kb in range(12, NKB)]
            feed += [("v", 2 + hh, kb) for hh in range(2) for kb in range(8)]
            feed += [("q", 6, pi) for pi in range(3)]
            feed += [("q", 7, pi) for pi in range(3)]
            feed += [("v", 2 + hh, kb) for hh in range(2) for kb in range(8, 12)]
            # b1 v transposes for kb 12-15 are only needed by qc3 — they are
            # held back as PE filler for the thin b1-qc0 stretch below
            late_vt = [("v", 2 + hh, kb) for hh in range(2) for kb in range(12, NKB)]

            run_chunk(attn_units_for_seg(0))              # b0 qc0-1
            coll_unit(0)
            run_chunk(attn_units_for_seg(1))              # b0 qc2-3
            coll_unit(1)
            drain_feed()  # all b1 qkv/vt must be emitted before b1 attention
            main2 = attn_units_for_seg(2)                 # b1 qc1-2
            for i, u in enumerate(main2):
                if i == 20:
                    post_dma(0)
                run_unit(u)
            main3 = attn_units_for_seg(3)                 # b1 qc0 (6 units)
            vt_i = 0
            for i, u in enumerate(main3):
                if i == 2:
                    post_dma(1)
                run_unit(u)
                for _ in range(2):
                    if vt_i < len(late_vt):
                        run_unit(late_vt[vt_i])
                        vt_i += 1
                if i == 3:
                    # seg0's projection (inputs landed ~60us ago, nothing
                    # collective-gated) fills this thin stretch so HAM does
                    # not re-throttle right before qc3's attention
                    proj_group(0)
            coll_unit(2)  # merged segs 2+3 exchange
            main4 = attn_units_for_seg(4)                 # b1 qc3 (18 units)
            for i, u in enumerate(main4):
                if i == 2:
                    post_dma(2)
                if i == 4:
                    post_dma(3)
                run_unit(u)
            coll_unit(3)
            post_dma(4)
            # tail: held-back projections fill the final collective's flight
            # time; the seg3 half of the merged 3+4 group runs as soon as the
            # merged A2A lands, leaving only the seg4 half + copy + store
            # gated on the final collective
            proj_group(1)
            proj_group(2)
            y34 = y_locs[3]
            ps34 = psp.tile(
                [128, C], F32, tag="st", bufs=3,
                padded_shape=[128, 1024], name="ps34",
            )
            for half in range(2):  # 0: seg3 (gated merged A2A), 1: seg4
                for mh in range(2):
                    for cb in range(NCB):
                        nc.tensor.matmul(
                            ps34[64 * half : 64 * half + 64,
                                 512 * mh : 512 * mh + 512],
                            lhsT=y34[:, cb, 64 * half : 64 * half + 64],
                            rhs=wo_sb[:, cb, 512 * mh : 512 * mh + 512],
                            start=(cb == 0),
                            stop=(cb == NCB - 1),
                        )
            o34 = work.tile([128, C], BF16, tag="osb", bufs=2, name="o34")
            nc.vector.tensor_copy(o34[:], ps34[:])
            nc.sync.dma_start(out[OUTOFF[3] : OUTOFF[3] + 128, :], o34[:])

    nc.compile()
    return nc


_NC = None


def _get_nc():
    global _NC
    if _NC is None:
        _NC = build_nc()
    return _NC


def _host_consts():
    idx = np.arange(128)
    mtri = np.where(idx[None, :] >= idx[:, None], 0.0, MASKVAL).astype(np.float32)
    ident = np.concatenate([np.eye(64, dtype=np.float32)] * 2, axis=0)
    return mtri.astype(NPBF16), ident.astype(NPBF16)


def _pack_w(wT):
    # [C, cols] -> [128, NCB*cols]: w_sb[p, cb*cols+j] = wT[128*cb+p, j]
    cols = wT.shape[1]
    return np.ascontiguousarray(
        wT.reshape(NCB, 128, cols).transpose(1, 0, 2).reshape(128, NCB * cols)
    )


def _make_in_maps(x, Wq, Wk, Wv, Wo):
    xT = np.ascontiguousarray(x.reshape(TF, C).T).astype(NPBF16)
    woT = _pack_w(Wo.T).astype(NPBF16)
    mtri, ident = _host_consts()
    in_maps = []
    for c in range(NCORES):
        rows = slice(CP * c, CP * c + CP)
        in_maps.append(
            {
                "xT": xT,
                "wqT": _pack_w(Wq[rows].T).astype(NPBF16),
                "wkT": _pack_w(Wk[rows].T).astype(NPBF16),
                "wvT": _pack_w(Wv[rows].T).astype(NPBF16),
                "woT": woT,
                "mtri": mtri,
                "ident": ident,
            }
        )
    return in_maps


def _assemble(results):
    full = np.zeros((TF, C), dtype=np.float32)
    for c in range(NCORES):
        o = results[c]["out"].astype(np.float32)
        for g in range(NSEG):
            b, qcs, tps = SEGS[g]
            cpq = 512 // tps  # chunks per q-chunk
            qc = qcs[c // cpq]
            base = 2048 * b + 512 * qc + tps * (c % cpq)
            full[base : base + tps] = o[OUTOFF[g] : OUTOFF[g] + tps]
    return full.reshape(B, T, C)


def kernel(x, mask, Wq, Wk, Wv, Wo):
    del mask  # causal mask is hardcoded in the device kernel
    in_maps = _make_in_maps(
        np.asarray(x, dtype=np.float32),
        np.asarray(Wq, dtype=np.float32),
        np.asarray(Wk, dtype=np.float32),
        np.asarray(Wv, dtype=np.float32),
        np.asarray(Wo, dtype=np.float32),
    )
    nc = _get_nc()
    res = run_bass_kernel_spmd(nc, in_maps, core_ids=list(range(NCORES)))
    return _assemble(res.results)


# revision 25
# speedup vs baseline: 1.0965x; 1.0472x over previous
"""Distributed causal multi-head attention for TRN2 (8 NeuronCores).

Sharding: tensor-parallel over heads — core c owns heads {2c, 2c+1} for both
batches. QKV projections computed in transposed layout (feature on partitions,
tokens on free axis), attention computed as S.T = K @ Q.T per 128-key block
with softmax denominators obtained by augmenting V with a ones column. Four
AllToAlls re-shard from head-parallel to token-parallel as attention
progresses; each core then applies the output projection for its 512 tokens.

Scheduling notes (hardware-measured):
- y is normalized on the SEND side: the eviction computes 1/den locally,
  broadcasts it across the 64 v-dim partitions with a stride-0 SBUF->SBUF
  DMA on sync, and the eviction copy becomes a multiply. The A2A ships
  normalized y (128 rows per chunk, no denominator row), so the post path
  is load -> projection with no DVE op downstream of any collective.
  The broadcast must NOT ride the gpsimd queue: collective_compute blocks
  that queue for the full trigger..completion flight (~15-30us), and an
  eviction ingredient parked there stalls the PE via the PSUM ot pool.
- Segs 2+3 (b1 qc1-2 and qc0) share one A2A: their evictions finish
  back-to-back and separate collectives serialize on the CC stream at a
  full flight (~15us) each.
- The two 64-token segments (b1 qc0 / qc3) write adjacent output rows
  (384..448..512); their projections share one PSUM tile, with the qc0
  half gated on the merged A2A so only the qc3 half + copy + store trail
  the final collective.
- x/weights land by need-time on the 3 DMA rings (sync/gpsimd/scalar,
  each ~50GB/s, ~FIFO): wave0+wq/wk -> prologue, wv -> v-proj (~11us),
  wave1 -> q1 (~18us), wave2a -> q2 (~26us), wave3a -> q4/q5 (~32us),
  wave2b -> q3 (~40us), wave3b -> q6/q7 (~55us), wo (tail) last on scalar.
  While q3's tranche is in flight the PE chews b1's first-half qkv.
- seg0's output projection runs inside the thin b1-qc0 stretch (its
  inputs landed ~60us earlier) so HAM does not re-throttle before qc3;
  the b1 kb12-15 v transposes are held back for the same purpose.
- No warmup collective: the CC path has an absolute ~74us cold-start, so
  a warmup A2A only pushes the real A2A#0 later (measured +11us).
"""

import sys

sys.path.insert(0, "/opt/trn_rl_repo")

import numpy as np
import ml_dtypes

import concourse.bacc as bacc
import concourse.bass as bass
import concourse.mybir as mybir
import concourse.tile as tile
from concourse.bass_utils import run_bass_kernel_spmd

BF16 = mybir.dt.bfloat16
F32 = mybir.dt.float32
NPBF16 = ml_dtypes.bfloat16

B, T, C, H, D = 2, 2048, 1024, 16, 64
NCORES = 8
HPC = H // NCORES          # heads per core = 2
CP = HPC * D               # feature columns per core = 128
TF = B * T                 # flat tokens = 4096
TS = TF // NCORES          # output tokens per core = 512
# segments: (batch, qcs, tokens-per-core); b1's qc0 gets its own small A2A
# that triggers ~20us before attention ends, so the final barrier carries
# only qc3 and its post work is merged with qc0's (adjacent output rows)
SEGS = [
    (0, (0, 1), 128),
    (0, (2, 3), 128),
    (1, (1, 2), 128),
    (1, (0,), 64),
    (1, (3,), 64),
]
NSEG = len(SEGS)
OUTOFF = [0, 128, 256, 384, 448]
SEG_OF = {}
for _g, (_b, _qcs, _tps) in enumerate(SEGS):
    for _i, _qc in enumerate(_qcs):
        SEG_OF[(_b, _qc)] = (_g, _i, _tps)
NCB = C // 128             # feature blocks = 8
NQC = T // 512             # q-chunks per batch = 4
NKB = T // 128             # key blocks per batch = 16
SCALE = float(D) ** -0.5
MASKVAL = -30000.0
CH = 128                   # a2a chunk rows: 64 per head half, normalized y


def build_nc():
    nc = bacc.Bacc("TRN2", target_bir_lowering=False, num_devices=NCORES)

    xT = nc.dram_tensor("xT", [C, TF], BF16, kind="ExternalInput")
    # weights pre-packed on host to the on-chip layout [128, NCB, blockcols]
    wqT = nc.dram_tensor("wqT", [128, NCB * CP], BF16, kind="ExternalInput")
    wkT = nc.dram_tensor("wkT", [128, NCB * CP], BF16, kind="ExternalInput")
    wvT = nc.dram_tensor("wvT", [128, NCB * CP], BF16, kind="ExternalInput")
    woT = nc.dram_tensor("woT", [128, NCB * C], BF16, kind="ExternalInput")
    mtri = nc.dram_tensor("mtri", [128, 128], BF16, kind="ExternalInput")
    ident = nc.dram_tensor("ident", [128, 64], BF16, kind="ExternalInput")
    out = nc.dram_tensor("out", [TS, C], BF16, kind="ExternalOutput")

    with tile.TileContext(nc) as tc:
        with (
            tc.tile_pool(name="consts", bufs=1) as consts,
            tc.tile_pool(name="xp", bufs=1) as xp,
            tc.tile_pool(name="qkv", bufs=1) as qkv,
            tc.tile_pool(name="work", bufs=1) as work,
            tc.tile_pool(name="ps", bufs=1, space="PSUM") as psp,
            tc.tile_pool(name="dram", bufs=1, space="DRAM") as dram,
        ):
            # ---- weights & constants ----
            wq_sb = consts.tile([128, NCB, CP], BF16)
            wk_sb = consts.tile([128, NCB, CP], BF16)
            wv_sb = consts.tile([128, NCB, CP], BF16)
            wo_sb = consts.tile([128, NCB, C], BF16)
            mtri_sb = consts.tile([128, 128], BF16)
            ident_sb = consts.tile([128, 64], BF16)
            x_sb = [xp.tile([128, TF], BF16, name=f"x_sb{cb}") for cb in range(NCB)]
            hw_ = NCB // 2

            def wdma(eng, w_t, w_d, lo, hi):
                eng.dma_start(w_t[:, lo:hi, :], w_d[:, lo * CP : hi * CP])

            def xdma(eng, cb, t0, t1):
                eng.dma_start(x_sb[cb][:, t0:t1], xT[128 * cb : 128 * cb + 128, t0:t1])

            # Only sync/scalar (HWDGE) and gpsimd issue DMAs. Per-ring
            # transfers serialize at ~50GB/s, so chunks are placed by
            # need-time: wave0 (tokens 0-511) + wq/wk feed the prologue,
            # wave1 (512-1023) the q1 units (~15us), wave2 (1024-2047) the
            # q2 units (~28us), wave3a/b (b1 halves) the q4/q6 units
            # (~75/110us), wo the tail (~190us). The sync ring carries only
            # ~0.9MB so eviction DMAs from ~30us are never queued behind bulk.
            _q3 = (nc.sync, nc.gpsimd, nc.scalar)
            nc.scalar.dma_start(mtri_sb[:], mtri[:])
            nc.gpsimd.dma_start(ident_sb[:], ident[:])
            wdma(nc.sync, wq_sb, wqT, 0, hw_)
            wdma(nc.gpsimd, wq_sb, wqT, hw_, NCB)
            wdma(nc.scalar, wk_sb, wkT, 0, hw_)
            for cb in (0, 1, 2, 3, 4, 5):
                xdma(_q3[cb % 3], cb, 0, 512)
            wdma(nc.scalar, wk_sb, wkT, hw_, NCB)
            xdma(nc.sync, 6, 0, 512)
            xdma(nc.gpsimd, 7, 0, 512)
            # wv right after wave0: the prologue v-proj needs it ~11us
            wdma(nc.sync, wv_sb, wvT, 0, hw_)
            wdma(nc.gpsimd, wv_sb, wvT, hw_, NCB)
            # Wave 1: x tokens 512-1023
            for cb in range(NCB):
                xdma(_q3[cb % 3], cb, 512, 1024)
            # Waves 2a / 3a / 2b / 3b interleaved by feed consumption order
            # (q2 ~26us, q4/q5 ~32us, q3 ~40us, q6/q7 ~55us): while q3's
            # tranche is still in flight the PE chews the b1 first-half qkv
            for cb in range(NCB):
                xdma(_q3[cb % 3], cb, 1024, 1536)
            for cb in range(NCB):
                xdma(_q3[(cb + 1) % 3], cb, 1536, 2048)
            for cb in range(NCB):
                xdma(_q3[(cb + 2) % 3], cb, 2048, 3072)
            for cb in range(NCB):
                xdma(_q3[cb % 3], cb, 3072, 4096)
            # wo rides the scalar ring (its issues all precede the first exp,
            # and its transfers contend with nothing the PE waits on) so the
            # gpsimd ring is clear for the post y loads by ~85us
            nc.scalar.dma_start(wo_sb[:], woT[:])

            qT_sb = qkv.tile([128, TF], BF16)
            kT_sb = qkv.tile([128, TF], BF16)
            vT_sb = qkv.tile([128, TF], BF16)
            projs = ((wq_sb, qT_sb), (wk_sb, kT_sb), (wv_sb, vT_sb))

            v_sb = [work.tile([128, NKB, 65], BF16, name=f"v_sb{p}") for p in range(4)]

            # A2A groups: segs 2+3 share one collective (their evictions
            # complete back-to-back, and a separate seg3 A2A would serialize
            # behind seg2's on the CC stream, adding a full ~15us flight)
            GRP_W = [128, 128, 192, 64]        # chunk cols per group
            SEG_GRP = {0: (0, 0), 1: (1, 0), 2: (2, 0), 3: (2, 128), 4: (3, 0)}
            a2a_in = [
                dram.tile([NCORES * CH, GRP_W[g]], BF16, name=f"a2a_in{g}")
                for g in range(4)
            ]
            a2a_out = [
                dram.tile([NCORES * CH, GRP_W[g]], BF16, name=f"a2a_out{g}")
                for g in range(4)
            ]

            # ---------- emission units ----------
            def qkv_unit(tcn, pi):
                # 512-token units: each gates on exactly one x tranche DMA
                w_sb, oT = projs[pi]
                t0 = 512 * tcn
                ps = psp.tile(
                    [128, 512], F32, tag="st", bufs=3,
                    padded_shape=[128, 1024], name="ps_proj",
                )
                for cb in range(NCB):
                    nc.tensor.matmul(
                        ps[:],
                        lhsT=w_sb[:, cb, :],
                        rhs=x_sb[cb][:, t0 : t0 + 512],
                        start=(cb == 0),
                        stop=(cb == NCB - 1),
                    )
                if pi == 1:
                    nc.scalar.copy(oT[:, t0 : t0 + 512], ps[:])
                else:
                    nc.vector.tensor_copy(oT[:, t0 : t0 + 512], ps[:])

            def vt_unit(pair, kb):
                hh, b = pair % 2, pair // 2
                if kb == 0:
                    nc.vector.memset(v_sb[pair][:, :, 64:65], 1.0)
                t0 = 2048 * b + 128 * kb
                vt_ps = psp.tile([128, 64], BF16, tag="ot", bufs=2, name="vt_ps")
                nc.tensor.transpose(
                    vt_ps[:],
                    vT_sb[64 * hh : 64 * hh + 64, t0 : t0 + 128],
                    ident_sb[64 * hh : 64 * hh + 64, :],
                )
                nc.vector.tensor_copy(v_sb[pair][:, kb, 0:64], vt_ps[:])

            # attention state per (pair, qc), lives across kbp units
            attn_ot = {}
            pending_pv = []

            def emit_pv(job):
                pair, qc, kbp, pT, offs = job
                ot = attn_ot[(pair, qc)]
                n_kb = 4 * qc + 4
                for h2 in range(2):
                    kb = 2 * kbp + h2
                    off = offs[h2]
                    nc.tensor.matmul(
                        ot[:, off:512],
                        lhsT=v_sb[pair][:, kb, :],
                        rhs=pT[:, 512 * h2 + off : 512 * h2 + 512],
                        start=(kb == 0),
                        stop=(kb == n_kb - 1),
                    )

            def drain_pending(pair=None, qc=None):
                rest = []
                for job in pending_pv:
                    if pair is None or (job[0] == pair and job[1] == qc):
                        emit_pv(job)
                    else:
                        rest.append(job)
                pending_pv[:] = rest

            def attn_unit(pair, qc, kbp):
                hh, b = pair % 2, pair // 2
                hs = slice(64 * hh, 64 * hh + 64)
                tb0 = 2048 * b
                q0 = tb0 + 512 * qc
                if kbp == 0:
                    attn_ot[(pair, qc)] = psp.tile(
                        [65, 512], F32, tag="ot", bufs=2, name="ot_ps"
                    )
                st = psp.tile([128, 1024], F32, tag="st", bufs=3, name="st_ps")
                offs = []
                for h2 in range(2):
                    kb = 2 * kbp + h2
                    off = max(0, 128 * kb - 512 * qc)
                    offs.append(off)
                    nc.tensor.matmul(
                        st[:, 512 * h2 + off : 512 * h2 + 512],
                        lhsT=kT_sb[hs, tb0 + 128 * kb : tb0 + 128 * kb + 128],
                        rhs=qT_sb[hs, q0 + off : q0 + 512],
                        start=True,
                        stop=True,
                    )
                for h2 in range(2):
                    kb = 2 * kbp + h2
                    if 128 * kb >= 512 * qc:  # diagonal block: triangular mask
                        off = offs[h2]
                        dd = slice(512 * h2 + off, 512 * h2 + off + 128)
                        nc.vector.tensor_add(st[:, dd], st[:, dd], mtri_sb[:])
                pT = work.tile([128, 1024], BF16, tag="pT", bufs=8, name="pT")
                o0 = offs[0]
                nc.scalar.activation(
                    pT[:, o0:1024],
                    st[:, o0:1024],
                    mybir.ActivationFunctionType.Exp,
                    scale=SCALE,
                )
                pending_pv.append((pair, qc, kbp, pT, offs))
                if len(pending_pv) > 3:
                    emit_pv(pending_pv.pop(0))

            def evict_unit(pair, qc):
                drain_pending(pair, qc)
                hh, b = pair % 2, pair // 2
                g, qi, tps = SEG_OF[(b, qc)]
                nch = 512 // tps          # chunks this eviction covers
                s0 = nch * qi
                ot = attn_ot.pop((pair, qc))
                # send-side normalization: reciprocal of the denominator row,
                # broadcast across the 64 v-dims, folded into the eviction
                # copy. All inputs are local (PSUM), so nothing anywhere is
                # gated on a collective, and the post-A2A path needs no DVE.
                den32 = work.tile([1, 512], F32, tag="den32", bufs=3, name="den32")
                rec32 = work.tile([1, 512], F32, tag="rec32", bufs=3, name="rec32")
                rec_dr = dram.tile([1, 512], F32, tag="recdr", bufs=3, name="rec_dr")
                recbc = work.tile([64, 512], F32, tag="recbc", bufs=2, name="recbc")
                y_sb = work.tile([64, 512], BF16, tag="y", bufs=12, name="y_sb")
                # stage through SBUF: the approx's bit-level seed needs raw
                # IEEE fp32, so don't feed it PSUM directly
                nc.vector.tensor_copy(den32[:], ot[64:65, :])
                nc.vector.reciprocal_approx_fast(rec32[:], den32[:])
                # broadcast 1/den across the 64 v-dim partitions via a tiny
                # DRAM bounce with a stride-0 source AP, both DMAs on sync:
                # the gpsimd queue blocks for each collective's full flight
                # (trigger..completion), and an eviction ingredient parked
                # there stalls the PE via the PSUM ot pool for tens of us.
                # (SBUF sources reject stride-0 partition dims; DRAM is fine.)
                nc.sync.dma_start(rec_dr[:], rec32[:])
                _r = rec_dr[:]
                rec_bc_src = bass.AP(_r.tensor, _r.offset, [[0, 64], [1, 512]])
                nc.sync.dma_start(recbc[:], rec_bc_src)
                nc.vector.tensor_mul(y_sb[:], ot[0:64, :], recbc[:])
                grp, coff = SEG_GRP[g]
                w = GRP_W[grp]
                ydst = bass.AP(
                    a2a_in[grp].tensor,
                    (s0 * CH + 64 * hh) * w + coff,
                    [[w, 64], [CH * w, nch], [1, tps]],
                )
                nc.sync.dma_start(ydst, y_sb[:, :])

            def coll_unit(grp):
                nc.gpsimd.collective_compute(
                    "AllToAll",
                    mybir.AluOpType.bypass,
                    replica_groups=[list(range(NCORES))],
                    ins=[a2a_in[grp][:].opt()],
                    outs=[a2a_out[grp][:].opt()],
                )

            # post y tiles: segs 0-2 individually; segs 3+4 share one tile
            # (adjacent 64-token column halves -> one 128-token proj group)
            y_locs = {}

            def post_dma(g):
                # gpsimd queue: these loads gate on the seg's A2A completing
                tps = SEGS[g][2]
                grp, coff = SEG_GRP[g]
                w = GRP_W[grp]
                if g < 3:
                    y_loc = work.tile(
                        [128, NCB, tps], BF16, tag="yloc", bufs=3, name="y_loc"
                    )
                    dst0 = 0
                    y_locs[g] = y_loc
                else:
                    if 3 not in y_locs:
                        y_locs[3] = work.tile(
                            [128, NCB, 128], BF16, tag="yloc34", bufs=1, name="y_loc34"
                        )
                    y_loc = y_locs[3]
                    dst0 = 64 * (g - 3)
                ysrc = bass.AP(
                    a2a_out[grp].tensor,
                    coff,
                    [[w, 128], [CH * w, NCB], [1, tps]],
                )
                nc.gpsimd.dma_start(y_loc[:, :, dst0 : dst0 + tps], ysrc)

            def proj_group(g):
                # g in {0,1,2}: 128 tokens at OUTOFF[g]; g==3: merged segs 3+4
                y_loc = y_locs[g]
                r0 = OUTOFF[g] if g < 3 else OUTOFF[3]
                ps = psp.tile(
                    [128, C], F32, tag="st", bufs=3,
                    padded_shape=[128, 1024], name="ps_op",
                )
                for mh in range(2):  # matmul out is one PSUM bank
                    for cb in range(NCB):
                        nc.tensor.matmul(
                            ps[:, 512 * mh : 512 * mh + 512],
                            lhsT=y_loc[:, cb, :],
                            rhs=wo_sb[:, cb, 512 * mh : 512 * mh + 512],
                            start=(cb == 0),
                            stop=(cb == NCB - 1),
                        )
                o_sb = work.tile([128, C], BF16, tag="osb", bufs=2, name="o_sb")
                nc.vector.tensor_copy(o_sb[:], ps[:])
                nc.sync.dma_start(out[r0 : r0 + 128, :], o_sb[:])

            def attn_units_for_seg(g):
                b, qcs, _ = SEGS[g]
                units = []
                for qc in qcs:
                    for hh in range(2):
                        pair = 2 * b + hh
                        for kbp in range(2 * qc + 2):
                            units.append(("a", pair, qc, kbp))
                        units.append(("e", pair, qc))
                return units

            def run_unit(u):
                if u[0] == "a":
                    attn_unit(u[1], u[2], u[3])
                elif u[0] == "e":
                    evict_unit(u[1], u[2])
                elif u[0] == "q":
                    qkv_unit(u[1], u[2])
                elif u[0] == "v":
                    vt_unit(u[1], u[2])

            feed = []
            fi = 0

            def run_chunk(main):
                nonlocal fi
                for u in main:
                    run_unit(u)
                    if fi < len(feed):
                        run_unit(feed[fi])
                        fi += 1

            def run_feed_one():
                nonlocal fi
                if fi < len(feed):
                    run_unit(feed[fi])
                    fi += 1

            def drain_feed():
                nonlocal fi
                while fi < len(feed):
                    run_unit(feed[fi])
                    fi += 1

            # ---------- schedule ----------
            # minimal prologue: only what attention (b0, qc0) needs up-front
            for pi in range(3):
                qkv_unit(0, pi)
            for hh in range(2):
                for kb in range(4):
                    vt_unit(hh, kb)

            # Phase feeds: a feed unit whose x tranche has not landed
            # head-of-line blocks the PE queue between attention units, so
            # each phase interleaves only work whose inputs land in time.
            feedA = [("q", 1, pi) for pi in range(3)]
            feedA += [("v", hh, kb) for hh in range(2) for kb in range(4, 8)]
            feedC = [("v", 0, kb) for kb in range(8, 12)]
            feedC += [("v", 1, kb) for kb in range(8, 12)]
            feedC += [("q", 3, pi) for pi in range(3)]
            feedC += [("q", 4, pi) for pi in range(3)]
            feedC += [("v", 0, kb) for kb in range(12, NKB)]
            feedC += [("q", 5, pi) for pi in range(3)]
            feedC += [("v", 1, kb) for kb in range(12, NKB)]
            feedC += [("v", 2, kb) for kb in range(7)]
            feedD = [("v", 2, kb) for kb in range(7, 8)]
            feedD += [("v", 3, kb) for kb in range(8)]
            feedD += [("q", 6, pi) for pi in range(3)]
            feedD += [("q", 7, pi) for pi in range(3)]
            feedD += [("v", 2 + hh, kb) for hh in range(2) for kb in range(8, 12)]
            late_vt = [("v", 2 + hh, kb) for hh in range(2) for kb in range(12, NKB)]

            def run_interleaved(main, fl):
                fj = 0
                for u in main:
                    run_unit(u)
                    if fj < len(fl):
                        run_unit(fl[fj])
                        fj += 1

            run_interleaved(attn_units_for_seg(0), feedA)  # b0 qc0-1
            coll_unit(0)
            for pi in range(3):  # q2 gates seg1's first attention unit
                qkv_unit(2, pi)
            run_interleaved(attn_units_for_seg(1), feedC)  # b0 qc2-3
            coll_unit(1)
            for u in feedD:  # all remaining b1 qkv/vt before b1 attention
                run_unit(u)
            main2 = attn_units_for_seg(2)                 # b1 qc1-2
            for i, u in enumerate(main2):
                if i == 20:
                    post_dma(0)
                run_unit(u)
            main3 = attn_units_for_seg(3)                 # b1 qc0 (6 units)
            vt_i = 0
            for i, u in enumerate(main3):
                if i == 2:
                    post_dma(1)
                run_unit(u)
                for _ in range(2):
                    if vt_i < len(late_vt):
                        run_unit(late_vt[vt_i])
                        vt_i += 1
                if i == 3:
                    # seg0's projection (inputs landed ~60us ago, nothing
                    # collective-gated) fills this thin stretch so HAM does
                    # not re-throttle right before qc3's attention
                    proj_group(0)
            coll_unit(2)  # merged segs 2+3 exchange
            main4 = attn_units_for_seg(4)                 # b1 qc3 (18 units)
            for i, u in enumerate(main4):
                if i == 2:
                    post_dma(2)
                if i == 4:
                    post_dma(3)
                run_unit(u)
            coll_unit(3)
            post_dma(4)
            # tail: held-back projections fill the final collective's flight
            # time; the seg3 half of the merged 3+4 group runs as soon as the
            # merged A2A lands, leaving only the seg4 half + copy + store
            # gated on the final collective
            proj_group(1)
            proj_group(2)
            y34 = y_locs[3]
            ps34 = psp.tile(
                [128, C], F32, tag="st", bufs=3,
                padded_shape=[128, 1024], name="ps34",
            )
            for half in range(2):  # 0: seg3 (gated merged A2A), 1: seg4
                for mh in range(2):
                    for cb in range(NCB):
                        nc.tensor.matmul(
                            ps34[64 * half : 64 * half + 64,
                                 512 * mh : 512 * mh + 512],
                            lhsT=y34[:, cb, 64 * half : 64 * half + 64],
                            rhs=wo_sb[:, cb, 512 * mh : 512 * mh + 512],
                            start=(cb == 0),
                            stop=(cb == NCB - 1),
                        )
            o34 = work.tile([128, C], BF16, tag="osb", bufs=2, name="o34")
            nc.vector.tensor_copy(o34[:], ps34[:])
            nc.sync.dma_start(out[OUTOFF[3] : OUTOFF[3] + 128, :], o34[:])

    nc.compile()
    return nc


_NC = None


def _get_nc():
    global _NC
    if _NC is None:
        _NC = build_nc()
    return _NC


def _host_consts():
    idx = np.arange(128)
    mtri = np.where(idx[None, :] >= idx[:, None], 0.0, MASKVAL).astype(np.float32)
    ident = np.concatenate([np.eye(64, dtype=np.float32)] * 2, axis=0)
    return mtri.astype(NPBF16), ident.astype(NPBF16)


def _pack_w(wT):
    # [C, cols] -> [128, NCB*cols]: w_sb[p, cb*cols+j] = wT[128*cb+p, j]
    cols = wT.shape[1]
    return np.ascontiguousarray(
        wT.reshape(NCB, 128, cols).transpose(1, 0, 2).reshape(128, NCB * cols)
    )


def _make_in_maps(x, Wq, Wk, Wv, Wo):
    xT = np.ascontiguousarray(x.reshape(TF, C).T).astype(NPBF16)
    woT = _pack_w(Wo.T).astype(NPBF16)
    mtri, ident = _host_consts()
    in_maps = []
    for c in range(NCORES):
        rows = slice(CP * c, CP * c + CP)
        in_maps.append(
            {
                "xT": xT,
                "wqT": _pack_w(Wq[rows].T).astype(NPBF16),
                "wkT": _pack_w(Wk[rows].T).astype(NPBF16),
                "wvT": _pack_w(Wv[rows].T).astype(NPBF16),
                "woT": woT,
                "mtri": mtri,
                "ident": ident,
            }
        )
    return in_maps


def _assemble(results):
    full = np.zeros((TF, C), dtype=np.float32)
    for c in range(NCORES):
        o = results[c]["out"].astype(np.float32)
        for g in range(NSEG):
            b, qcs, tps = SEGS[g]
            cpq = 512 // tps  # chunks per q-chunk
            qc = qcs[c // cpq]
            base = 2048 * b + 512 * qc + tps * (c % cpq)
            full[base : base + tps] = o[OUTOFF[g] : OUTOFF[g] + tps]
    return full.reshape(B, T, C)


def kernel(x, mask, Wq, Wk, Wv, Wo):
    del mask  # causal mask is hardcoded in the device kernel
    in_maps = _make_in_maps(
        np.asarray(x, dtype=np.float32),
        np.asarray(Wq, dtype=np.float32),
        np.asarray(Wk, dtype=np.float32),
        np.asarray(Wv, dtype=np.float32),
        np.asarray(Wo, dtype=np.float32),
    )
    nc = _get_nc()
    res = run_bass_kernel_spmd(nc, in_maps, core_ids=list(range(NCORES)))
    return _assemble(res.results)
